# revision 12
# baseline (speedup 1.0000x reference)
"""GPT (4-layer, E=768, H=12, T=1024, B=2, V=50257) forward on 8 trn2 cores.

Sharding:
  - Residual stream x token-sharded: core c owns tokens [c*256,(c+1)*256) of the
    flattened [2048] (batch-major), so cores 0-3 = batch 0, cores 4-7 = batch 1.
  - LN / MLP / residual adds fully token-local.
  - Attention head-sharded within each batch group of 4 cores (3 heads each):
    AllGather (transposed) hidden states per layer, compute q/k/v + attention +
    out-proj partials locally, ReduceScatter back to token shards.
  - lm_head vocab-sharded: final AllGather of lnf(x), each core computes
    [2048, 6284] logit slice (V padded 50257 -> 50272 = 8*6284).
  - All matmuls fp32r (fp32 data, full PE rate at moving-dim >= 256).
"""

import sys
from contextlib import ExitStack
import numpy as np

sys.path.insert(0, "/opt/trn_rl_repo")

import concourse.bass as bass
import concourse.mybir as mybir
import concourse.tile as tile
from concourse import bacc
from concourse.bass_utils import run_bass_kernel_spmd
from concourse.masks import make_identity

L, H, E, T, V = 4, 12, 768, 1024, 50257
B = 2
NC = 8
TS = (B * T) // NC          # 256 tokens per core
VS = 6284                   # vocab slice per core, even (padded V = 50272)
VPAD = VS * NC
HD = 64
NHC = 3                     # heads per core
EPS = 1e-5
SCALE = float(1.0 / np.sqrt(np.float32(E)))
F32 = mybir.dt.float32
F32R = mybir.dt.float32r

_CACHE = {}


def _build_program():
    nc = bacc.Bacc("TRN2", target_bir_lowering=False, debug=False, num_devices=NC)

    # ---- I/O -------------------------------------------------------------
    x0s = nc.dram_tensor("x0s", [TS, E], F32, kind="ExternalInput")
    wqk = nc.dram_tensor("wqk", [L, E, 2 * NHC * HD], F32, kind="ExternalInput")
    bqk = nc.dram_tensor("bqk", [L, 6, 64], F32, kind="ExternalInput")
    wv = nc.dram_tensor("wv", [L, E, 256], F32, kind="ExternalInput")
    bv = nc.dram_tensor("bv", [L, 3, HD], F32, kind="ExternalInput")
    watp = nc.dram_tensor("watp", [L, NHC * HD, E], F32, kind="ExternalInput")
    atpb = nc.dram_tensor("atpb", [L, E], F32, kind="ExternalInput")
    fcw = nc.dram_tensor("fcw", [L, 24, 6, 128, 128], F32, kind="ExternalInput")
    fcb = nc.dram_tensor("fcb", [L, 24, 128], F32, kind="ExternalInput")
    prw = nc.dram_tensor("prw", [L, 4 * E, E], F32, kind="ExternalInput")
    prb = nc.dram_tensor("prb", [L, E], F32, kind="ExternalInput")
    ln1g = nc.dram_tensor("ln1g", [L, E], F32, kind="ExternalInput")
    ln1b = nc.dram_tensor("ln1b", [L, E], F32, kind="ExternalInput")
    ln2g = nc.dram_tensor("ln2g", [L, E], F32, kind="ExternalInput")
    ln2b = nc.dram_tensor("ln2b", [L, E], F32, kind="ExternalInput")
    lnfg = nc.dram_tensor("lnfg", [1, E], F32, kind="ExternalInput")
    lnfb = nc.dram_tensor("lnfb", [1, E], F32, kind="ExternalInput")
    tri = nc.dram_tensor("tri", [128, 128], F32, kind="ExternalInput")
    wteT = nc.dram_tensor("wteT", [E, VS], F32, kind="ExternalInput")
    logits = nc.dram_tensor("logits", [B * T, VS], F32, kind="ExternalOutput")

    g_all = [list(range(NC))]
    g_batch = [[0, 1, 2, 3], [4, 5, 6, 7]]

    def bcast_row(pool, src_ap, n, dtype, w):
        """Replicate a [w] DRAM row across n partitions via broadcast DMA."""
        t = pool.tile([n, w], dtype)
        in_ap = bass.AP(
            tensor=src_ap.tensor,
            offset=src_ap.offset,
            ap=[[0, n]] + [list(p) for p in src_ap.ap],
        )
        if dtype == F32R:
            in_ap = in_ap.bitcast(F32R)
        nc.sync.dma_start(out=t[:], in_=in_ap)
        return t

    with tile.TileContext(nc) as tc, ExitStack() as es:
        const = es.enter_context(tc.tile_pool(name="const", bufs=1))
        xp = es.enter_context(tc.tile_pool(name="xp", bufs=1))
        lnrow = es.enter_context(tc.tile_pool(name="lnrow", bufs=3))
        stat = es.enter_context(tc.tile_pool(name="stat", bufs=4))
        hpool = es.enter_context(tc.tile_pool(name="hpool", bufs=2))
        hTp = es.enter_context(tc.tile_pool(name="hTp", bufs=1))
        dram = es.enter_context(tc.tile_pool(name="dram", bufs=1, space="DRAM"))
        psT = es.enter_context(tc.tile_pool(name="psT", bufs=1, space="PSUM"))


        ident_f = const.tile([128, 128], F32, name="ident_f")
        make_identity(nc, ident_f)
        ident = const.tile([128, 128], F32R, name="ident")
        nc.vector.tensor_copy(ident[:], ident_f[:])
        ones_f = const.tile([128, 1], F32, name="ones_f")
        nc.vector.memset(ones_f, 1.0)
        ones_r = const.tile([128, 1], F32R, name="ones_r")
        nc.vector.tensor_copy(ones_r[:], ones_f[:])
        tri_sb = const.tile([128, 128], F32R)
        nc.sync.dma_start(out=tri_sb[:], in_=tri[:, :].bitcast(F32R))
        eps_sb = const.tile([128, 1], F32)
        nc.vector.memset(eps_sb, EPS)

        # persistent residual stream [256, 768] as two [128, 768] tiles
        x_sb = [xp.tile([128, E], F32, tag=f"x{t}", name=f"x{t}") for t in range(2)]
        for t in range(2):
            nc.sync.dma_start(out=x_sb[t][:], in_=x0s[t * 128:(t + 1) * 128, :])

        # DRAM bounce buffers for collectives
        hT_in = dram.tile([E, TS], F32)
        hT_ag = dram.tile([NC * E, TS], F32)
        rs_in = dram.tile([T, E], mybir.dt.bfloat16)
        rs_out = dram.tile([TS, E], mybir.dt.bfloat16)
        xf_in = dram.tile([E, TS], F32)
        xf_ag = dram.tile([NC * E, TS], F32)

        def layernorm_t(x_ap, g_bc, b_bc, out_tile):
            """LN over free dim (768) of [128, 768] tile -> out (f32r)."""
            stats = stat.tile([128, 3, 6], F32, tag="bn_stats", name="bn_stats_t")
            xr = x_ap.rearrange("p (s d) -> p s d", s=3)
            for s in range(3):
                nc.vector.bn_stats(out=stats[:, s, :], in_=xr[:, s, :])
            mv = stat.tile([128, 2], F32, tag="bn_aggr", name="bn_aggr_t")
            nc.vector.bn_aggr(out=mv[:], in_=stats[:])
            rstd = stat.tile([128, 1], F32, tag="rstd", name="rstd_t")
            nc.scalar.activation(out=rstd[:], in_=mv[:, 1:2],
                                 func=mybir.ActivationFunctionType.Sqrt,
                                 bias=eps_sb[:], scale=1.0)
            nc.vector.reciprocal(out=rstd[:], in_=rstd[:])
            nc.vector.tensor_scalar(out=out_tile[:], in0=x_ap,
                                    scalar1=mv[:, 0:1], scalar2=rstd[:],
                                    op0=mybir.AluOpType.subtract,
                                    op1=mybir.AluOpType.mult)
            nc.vector.tensor_mul(out=out_tile[:], in0=out_tile[:], in1=g_bc[:])
            nc.vector.tensor_add(out=out_tile[:], in0=out_tile[:], in1=b_bc[:])

        def transpose_to(hsrc, dst_tiles, dst_col):
            """hsrc [128,768] f32r -> dst_tiles[k][:, dst_col:dst_col+128]."""
            for k in range(6):
                pt = psT.tile([128, 128], F32R, tag="tr", name="tr")
                nc.tensor.transpose(pt[:], hsrc[:, k * 128:(k + 1) * 128], ident[:])
                dst = dst_tiles[k][:, dst_col:dst_col + 128]
                if k % 2 == 0:
                    nc.vector.tensor_copy(dst, pt[:])
                else:
                    nc.scalar.activation(out=dst, in_=pt[:],
                                         func=mybir.ActivationFunctionType.Copy)

        # Attention AllGather runs over per-batch groups [[0..3],[4..7]], so
        # every core's AG output holds exactly its own batch's 4 rank-blocks
        # at rows [0, 3072) -- the program stays uniform across cores (SPMD).
        es_l = es.enter_context(ExitStack())
        hTbp = es_l.enter_context(tc.tile_pool(name="hTbp", bufs=1))
        wqkp = es_l.enter_context(tc.tile_pool(name="wqkp", bufs=1))
        wvp = es_l.enter_context(tc.tile_pool(name="wvp", bufs=1))
        watpp = es_l.enter_context(tc.tile_pool(name="watpp", bufs=1))
        bias_p = es_l.enter_context(tc.tile_pool(name="bias_p", bufs=2))
        qkTp = es_l.enter_context(tc.tile_pool(name="qkTp", bufs=1))
        vp = es_l.enter_context(tc.tile_pool(name="vp", bufs=1))
        ep = es_l.enter_context(tc.tile_pool(name="ep", bufs=3))
        yp = es_l.enter_context(tc.tile_pool(name="yp", bufs=1))
        sm = es_l.enter_context(tc.tile_pool(name="sm", bufs=1))
        fcwp = es_l.enter_context(tc.tile_pool(name="fcwp", bufs=4))
        mTp = es_l.enter_context(tc.tile_pool(name="mTp", bufs=1))
        prwp = es_l.enter_context(tc.tile_pool(name="prwp", bufs=3))

        def scope(name):
            sid, _ = nc.enter_named_scope(name, False)
            _SCOPES.append((name, sid))

        def unscope():
            name, sid = _SCOPES.pop()
            nc.leave_named_scope(name, sid, False)

        _SCOPES = []
        for layer in range(L):
            scope(f"L{layer}_ln1_ag")
            ln1g_bc = bcast_row(lnrow, ln1g[layer], 128, F32R, E)
            ln1b_bc = bcast_row(lnrow, ln1b[layer], 128, F32R, E)

            # ---- LN1 + transpose -> hT_sb [768, 256] ----
            hT_sb = [hTp.tile([128, TS], F32R, tag=f"hT{k}", name=f"hT{k}") for k in range(6)]
            for t in range(2):
                h_t = hpool.tile([128, E], F32R, tag="h", name="h")
                layernorm_t(x_sb[t][:], ln1g_bc, ln1b_bc, h_t)
                transpose_to(h_t, hT_sb, t * 128)
            for k in range(6):
                nc.sync.dma_start(out=hT_in[k * 128:(k + 1) * 128, :].bitcast(F32R),
                                  in_=hT_sb[k][:])

            # ---- AllGather hidden (within batch group of 4) ----
            nc.gpsimd.collective_compute(
                "AllGather", mybir.AluOpType.bypass,
                replica_groups=g_batch,
                ins=[hT_in.opt()],
                outs=[hT_ag[0:4 * E, :].opt()],
            )
            # load hT for my batch: 6 tiles [128, 1024]
            hTb = [hTbp.tile([128, T], F32R, tag=f"hTb{k}", name=f"hTb{k}") for k in range(6)]
            for k in range(6):
                for r in range(4):
                    nc.sync.dma_start(
                        out=hTb[k][:, r * TS:(r + 1) * TS],
                        in_=hT_ag[r * E + k * 128: r * E + (k + 1) * 128, :].bitcast(F32R))

            unscope()
            scope(f"L{layer}_qkv")
            # ---- QKV ----
            es_a = ExitStack()
            psQ = es_a.enter_context(tc.tile_pool(name="psQ", bufs=2, space="PSUM"))
            psV = es_a.enter_context(tc.tile_pool(name="psV", bufs=1, space="PSUM"))
            psS = es_a.enter_context(tc.tile_pool(name="psS", bufs=2, space="PSUM"))
            psY = es_a.enter_context(tc.tile_pool(name="psY", bufs=1, space="PSUM"))
            wqk_sb = [wqkp.tile([128, 384], F32R, tag=f"wqk{k}", name=f"wqk{k}") for k in range(6)]
            for k in range(6):
                nc.sync.dma_start(out=wqk_sb[k][:],
                                  in_=wqk[layer, k * 128:(k + 1) * 128, :].bitcast(F32R))
            bqk_sb2 = []
            for m in range(6):
                bt = bias_p.tile([64, 1], F32, tag=f"bqk{m}", name=f"bqk{m}")
                nc.sync.dma_start(out=bt[:], in_=bqk[layer, m].unsqueeze(1))
                bqk_sb2.append(bt)
            qkT = [qkTp.tile([64, T], F32R, tag=f"qkT{m}", name=f"qkT{m}") for m in range(6)]
            for m in range(6):
                for n in range(2):
                    ps = psQ.tile([64, 512], F32, tag="q", name="q")
                    for k in range(6):
                        nc.tensor.matmul(ps[:], wqk_sb[k][:, m * 64:(m + 1) * 64],
                                         hTb[k][:, n * 512:(n + 1) * 512],
                                         start=(k == 0), stop=(k == 5))
                    nc.vector.tensor_scalar_add(out=qkT[m][:, n * 512:(n + 1) * 512],
                                                in0=ps[:],
                                                scalar1=bqk_sb2[m][:, 0:1])

            wv_sb = [wvp.tile([128, 256], F32R, tag=f"wv{k}", name=f"wv{k}") for k in range(6)]
            for k in range(6):
                nc.sync.dma_start(out=wv_sb[k][:],
                                  in_=wv[layer, k * 128:(k + 1) * 128, :].bitcast(F32R))
            v_sb = [vp.tile([128, 3 * 65], F32R, tag=f"v{t}", name=f"v{t}") for t in range(8)]
            for t in range(8):
                ps = psV.tile([128, 256], F32, tag="v", name="v")
                for k in range(6):
                    nc.tensor.matmul(ps[:], hTb[k][:, t * 128:(t + 1) * 128],
                                     wv_sb[k][:], start=(k == 0), stop=(k == 5))
                for h in range(3):
                    nc.scalar.activation(out=v_sb[t][:, 65 * h:65 * h + 64],
                                         in_=ps[:, 64 * h:64 * h + 64],
                                         func=mybir.ActivationFunctionType.Copy)
                    nc.vector.tensor_copy(v_sb[t][:, 65 * h + 64:65 * h + 65],
                                          ones_r[:])

            bv_sb = bias_p.tile([64, 3], F32, tag="bv", name="bv")
            nc.sync.dma_start(out=bv_sb[:], in_=bv[layer].transpose([1, 0]))

            unscope()
            scope(f"L{layer}_attn")
            # ---- attention per head ----
            yT_sb = []
            for h in range(3):
                qT = qkT[h][:, :]
                kT = qkT[3 + h][:, :]
                yps = psY.tile([65, T], F32, tag="y", name="y")
                for j in range(8):
                    qs = j * 128
                    qlen = T - qs
                    e_sb = ep.tile([128, T], F32R, tag="e", name="e")
                    off = 0
                    while off < qlen:
                        cl = min(512, qlen - off)
                        pss = psS.tile([128, 512], F32, tag="s", name="s")
                        nc.tensor.matmul(pss[:, 0:cl], kT[:, j * 128:(j + 1) * 128],
                                         qT[:, qs + off: qs + off + cl],
                                         start=True, stop=True)
                        nc.scalar.activation(out=e_sb[:, off:off + cl],
                                             in_=pss[:, 0:cl],
                                             func=mybir.ActivationFunctionType.Exp,
                                             scale=SCALE)
                        off += cl
                    nc.vector.tensor_mul(out=e_sb[:, 0:128], in0=e_sb[:, 0:128],
                                         in1=tri_sb[:])
                    # accumulate yT over k-blocks, per psum bank
                    if qs < 512:
                        nc.tensor.matmul(yps[:, qs:512],
                                         v_sb[j][:, 65 * h:65 * h + 65],
                                         e_sb[:, 0:512 - qs],
                                         start=(j == 0), stop=(j == 3))
                    nc.tensor.matmul(yps[:, max(qs, 512):T],
                                     v_sb[j][:, 65 * h:65 * h + 65],
                                     e_sb[:, max(qs, 512) - qs:qlen],
                                     start=(j == 0), stop=(j == 7))
                recip = sm.tile([1, T], F32, tag="recip", name="recip")
                nc.vector.reciprocal(out=recip[:], in_=yps[64:65, :])
                recip_bc = sm.tile([64, T], F32, tag="recip_bc", name="recip_bc")
                nc.gpsimd.partition_broadcast(recip_bc[:], recip[:])
                yT = yp.tile([64, T], F32R, tag=f"yT{h}", name=f"yT{h}")
                nc.vector.tensor_mul(out=yT[:], in0=yps[0:64, :], in1=recip_bc[:])
                nc.vector.tensor_scalar_add(out=yT[:], in0=yT[:],
                                            scalar1=bv_sb[:, h:h + 1])
                yT_sb.append(yT)

            unscope()
            scope(f"L{layer}_proj_rs")
            # ---- out-proj partials -> ReduceScatter ----
            es_a.close()
            es_b = ExitStack()
            psO = es_b.enter_context(tc.tile_pool(name="psO", bufs=2, space="PSUM"))
            watp_sb = [watpp.tile([64, E], F32R, tag=f"watp{h}", name=f"watp{h}") for h in range(3)]
            for h in range(3):
                nc.sync.dma_start(out=watp_sb[h][:],
                                  in_=watp[layer, 64 * h:64 * h + 64, :].bitcast(F32R))
            for t in range(8):
                ps = psO.tile([128, E], F32, tag="o", name="o")
                for n0, n1 in ((0, 512), (512, 768)):
                    for h in range(3):
                        nc.tensor.matmul(ps[:, n0:n1],
                                         yT_sb[h][:, t * 128:(t + 1) * 128],
                                         watp_sb[h][:, n0:n1],
                                         start=(h == 0), stop=(h == 2))
                ao = hpool.tile([128, E], mybir.dt.bfloat16, tag="ao", name="ao")
                if t % 2 == 0:
                    nc.vector.tensor_copy(ao[:], ps[:])
                else:
                    nc.scalar.activation(out=ao[:], in_=ps[:],
                                         func=mybir.ActivationFunctionType.Copy)
                nc.sync.dma_start(out=rs_in[t * 128:(t + 1) * 128, :], in_=ao[:])
            nc.gpsimd.collective_compute(
                "ReduceScatter", mybir.AluOpType.add,
                replica_groups=g_batch,
                ins=[rs_in.opt()],
                outs=[rs_out.opt()],
            )
            atpb_bc = bcast_row(lnrow, atpb[layer], 128, F32, E)
            for t in range(2):
                rsb = hpool.tile([128, E], mybir.dt.bfloat16, tag="rsb", name="rsb")
                nc.sync.dma_start(out=rsb[:], in_=rs_out[t * 128:(t + 1) * 128, :])
                nc.vector.tensor_add(out=x_sb[t][:], in0=x_sb[t][:], in1=rsb[:])
                nc.vector.tensor_add(out=x_sb[t][:], in0=x_sb[t][:], in1=atpb_bc[:])

            unscope()
            scope(f"L{layer}_mlp")
            # ---- LN2 + transpose ----
            ln2g_bc = bcast_row(lnrow, ln2g[layer], 128, F32R, E)
            ln2b_bc = bcast_row(lnrow, ln2b[layer], 128, F32R, E)
            h2T = [hTp.tile([128, TS], F32R, tag=f"hT{k}", name=f"hT{k}") for k in range(6)]
            for t in range(2):
                h_t = hpool.tile([128, E], F32R, tag="h", name="h")
                layernorm_t(x_sb[t][:], ln2g_bc, ln2b_bc, h_t)
                transpose_to(h_t, h2T, t * 128)

            # ---- MLP fc (mT = gelu(fc_w.T @ h2T + fc_b)) ----

            es_b.close()
            es_c = ExitStack()
            psM = es_c.enter_context(tc.tile_pool(name="psM", bufs=2, space="PSUM"))
            psP = es_c.enter_context(tc.tile_pool(name="psP", bufs=1, space="PSUM"))
            fcb_sb = bias_p.tile([128, 24], F32, tag="fcb", name="fcb")
            nc.sync.dma_start(out=fcb_sb[:], in_=fcb[layer].transpose([1, 0]))
            mT = [mTp.tile([128, TS], F32R, tag=f"mT{m}", name=f"mT{m}") for m in range(24)]
            for m in range(24):
                ps = psM.tile([128, TS], F32, tag="m", name="m")
                for k in range(6):
                    fck = fcwp.tile([128, 128], F32R, tag=f"fck{k}", name=f"fck{k}")
                    nc.sync.dma_start(
                        out=fck[:],
                        in_=fcw[layer, m, k].bitcast(F32R))
                    nc.tensor.matmul(ps[:], fck[:],
                                     h2T[k][:], start=(k == 0), stop=(k == 5))
                nc.scalar.activation(out=mT[m][:], in_=ps[:],
                                     func=mybir.ActivationFunctionType.Gelu_apprx_tanh,
                                     bias=fcb_sb[:, m:m + 1])

            # ---- MLP pr + residual ----
            prb_bc = bcast_row(lnrow, prb[layer], 128, F32, E)
            ps2 = [psP.tile([128, E], F32, tag=f"p{t}", name=f"p{t}") for t in range(2)]
            for k in range(24):
                prw_sb = prwp.tile([128, E], F32R, tag="prw", name="prw")
                nc.sync.dma_start(out=prw_sb[:],
                                  in_=prw[layer, k * 128:(k + 1) * 128, :].bitcast(F32R))
                for t in range(2):
                    for n0, n1 in ((0, 512), (512, 768)):
                        nc.tensor.matmul(ps2[t][:, n0:n1],
                                         mT[k][:, t * 128:(t + 1) * 128],
                                         prw_sb[:, n0:n1],
                                         start=(k == 0), stop=(k == 23))
            for t in range(2):
                nc.vector.tensor_add(out=x_sb[t][:], in0=x_sb[t][:], in1=ps2[t][:])
                nc.vector.tensor_add(out=x_sb[t][:], in0=x_sb[t][:], in1=prb_bc[:])
            es_c.close()
            unscope()

        # ---- final LN + AllGather(all 8) + lm_head ----
        scope("lnf_ag")
        lnfg_bc = bcast_row(lnrow, lnfg[0], 128, F32R, E)
        lnfb_bc = bcast_row(lnrow, lnfb[0], 128, F32R, E)
        xfT = [hTp.tile([128, TS], F32R, tag=f"hT{k}", name=f"hT{k}") for k in range(6)]
        for t in range(2):
            h_t = hpool.tile([128, E], F32R, tag="h", name="h")
            layernorm_t(x_sb[t][:], lnfg_bc, lnfb_bc, h_t)
            transpose_to(h_t, xfT, t * 128)
        for k in range(6):
            nc.sync.dma_start(out=xf_in[k * 128:(k + 1) * 128, :].bitcast(F32R),
                              in_=xfT[k][:])
        nc.gpsimd.collective_compute(
            "AllGather", mybir.AluOpType.bypass,
            replica_groups=g_all,
            ins=[xf_in.opt()],
            outs=[xf_ag.opt()],
        )
        es_l.close()
        es_h = es.enter_context(ExitStack())
        xfp = es_h.enter_context(tc.tile_pool(name="xfp", bufs=1))
        wtep = es_h.enter_context(tc.tile_pool(name="wtep", bufs=3))
        psL = es_h.enter_context(tc.tile_pool(name="psL", bufs=4, space="PSUM"))

        xf_sb = [xfp.tile([128, TS], F32R, tag=f"xf{i}", name=f"xf{i}") for i in range(48)]
        for i in range(48):
            nc.sync.dma_start(out=xf_sb[i][:],
                              in_=xf_ag[i * 128:(i + 1) * 128, :].bitcast(F32R))
        unscope()

        scope("lmhead")
        nch = (VS + 511) // 512
        for n in range(nch):
            n0 = n * 512
            nw = min(512, VS - n0)
            wte_sb = [wtep.tile([128, 512], F32R, tag=f"wte{k}", name=f"wte{k}") for k in range(6)]
            for k in range(6):
                nc.sync.dma_start(out=wte_sb[k][:, 0:nw],
                                  in_=wteT[k * 128:(k + 1) * 128, n0:n0 + nw].bitcast(F32R))
            for t in range(16):
                r, half = t // 2, t % 2
                ps = psL.tile([128, 512], F32, tag="l", name="l")
                for k in range(6):
                    nc.tensor.matmul(ps[:, 0:nw],
                                     xf_sb[r * 6 + k][:, half * 128:(half + 1) * 128],
                                     wte_sb[k][:, 0:nw],
                                     start=(k == 0), stop=(k == 5))
                lo = wtep.tile([128, 512], F32, tag="lo", name="lo")
                if t % 2 == 0:
                    nc.vector.tensor_copy(lo[:, 0:nw], ps[:, 0:nw])
                else:
                    nc.scalar.activation(out=lo[:, 0:nw], in_=ps[:, 0:nw],
                                         func=mybir.ActivationFunctionType.Copy)
                nc.sync.dma_start(out=logits[t * 128:(t + 1) * 128, n0:n0 + nw],
                                  in_=lo[:, 0:nw])
        unscope()

    nc.compile()
    return nc


def _prep_inputs(idx, wte, wpe, ln1_w, ln1_b, attn_w, attn_b, atp_w, atp_b,
                 ln2_w, ln2_b, fc_w, fc_b, pr_w, pr_b, lnf_w, lnf_b):
    idx = np.asarray(idx)
    f = lambda a: np.ascontiguousarray(np.asarray(a), dtype=np.float32)
    wte, wpe = f(wte), f(wpe)
    x0 = wte[idx.reshape(-1)] + np.tile(wpe[:T], (B, 1))  # [2048, 768]
    wte_pad = np.zeros((VPAD, E), np.float32)
    wte_pad[:V] = wte
    wteT_full = np.ascontiguousarray(wte_pad.T)  # [768, VPAD]

    attn_w, attn_b = f(attn_w), f(attn_b)
    atp_w, atp_b = f(atp_w), f(atp_b)
    fc_w, fc_b, pr_w, pr_b = f(fc_w), f(fc_b), f(pr_w), f(pr_b)
    # [L, 768, 3072] -> [L, 24(m), 6(k), 128, 128] contiguous tiles
    fcw_tiled = np.ascontiguousarray(
        fc_w.reshape(L, 6, 128, 24, 128).transpose(0, 3, 1, 2, 4))
    tri = (np.arange(128)[None, :] >= np.arange(128)[:, None]).astype(np.float32)

    in_maps = []
    for c in range(NC):
        hs = 3 * (c % 4)
        qcols = [attn_w[:, :, h * HD:(h + 1) * HD] for h in range(hs, hs + 3)]
        kcols = [attn_w[:, :, E + h * HD:E + (h + 1) * HD] for h in range(hs, hs + 3)]
        vcols = [attn_w[:, :, 2 * E + h * HD:2 * E + (h + 1) * HD] for h in range(hs, hs + 3)]
        wqk_c = np.ascontiguousarray(np.concatenate(qcols + kcols, axis=2))
        wv_c = np.concatenate(vcols, axis=2)
        wv_c = np.ascontiguousarray(
            np.concatenate([wv_c, np.zeros((L, E, 64), np.float32)], axis=2))
        bq = [attn_b[:, h * HD:(h + 1) * HD] for h in range(hs, hs + 3)]
        bk = [attn_b[:, E + h * HD:E + (h + 1) * HD] for h in range(hs, hs + 3)]
        bvs = [attn_b[:, 2 * E + h * HD:2 * E + (h + 1) * HD] for h in range(hs, hs + 3)]
        bqk_c = np.concatenate(bq + bk, axis=1).reshape(L, 6, 64)
        bv_c = np.stack(bvs, axis=1)  # [L, 3, 64]
        watp_c = np.ascontiguousarray(atp_w[:, hs * HD:(hs + 3) * HD, :])
        in_maps.append({
            "x0s": np.ascontiguousarray(x0[c * TS:(c + 1) * TS]),
            "wqk": wqk_c, "bqk": np.ascontiguousarray(bqk_c),
            "wv": wv_c, "bv": np.ascontiguousarray(bv_c),
            "watp": watp_c, "atpb": atp_b,
            "fcw": fcw_tiled, "fcb": np.ascontiguousarray(fc_b.reshape(L, 24, 128)),
            "prw": pr_w, "prb": pr_b,
            "ln1g": f(ln1_w), "ln1b": f(ln1_b),
            "ln2g": f(ln2_w), "ln2b": f(ln2_b),
            "lnfg": f(lnf_w).reshape(1, E), "lnfb": f(lnf_b).reshape(1, E),
            "tri": tri,
            "wteT": np.ascontiguousarray(wteT_full[:, c * VS:(c + 1) * VS]),
        })
    return in_maps


def kernel(trace=False, **inputs):
    if "nc" not in _CACHE:
        _CACHE["nc"] = _build_program()
    nc = _CACHE["nc"]
    in_maps = _prep_inputs(**inputs)
    res = run_bass_kernel_spmd(nc, in_maps, core_ids=list(range(NC)), trace=trace)
    _CACHE["last_result"] = res
    logits = np.concatenate([res.results[c]["logits"] for c in range(NC)], axis=1)
    return logits[:, :V].reshape(B, T, V).astype(np.float32)



# revision 21
# speedup vs baseline: 1.3498x; 1.3498x over previous
"""GPT (4-layer, E=768, H=12, T=1024, B=2, V=50257) forward on 8 trn2 cores.

Sharding (v3):
  - Token-parallel residual: core c owns tokens [r*256,(r+1)*256) of batch
    c//4 (r = c%4).  LN / QKV / out-proj / MLP fully token-local.
  - Attention: per layer two AllGathers per batch group of 4 — K (dim-major
    [768,256] bf16) and V (token-major [256,780] bf16, softmax ones column
    pre-packed).  Each core computes all 12 heads for its own 256 queries
    against the full 1024 keys with a per-core causal mask (uniform SPMD
    program; masking by data, not control flow).  No ReduceScatter.
  - lm_head vocab-sharded: final AllGather of lnf(x) (transposed, bf16),
    each core computes a [2048, 6284] logit slice (V padded to 50272).
  - All matmul operands bf16 (fp32 PSUM accumulation); weights shipped bf16
    to halve HBM traffic and PE weight-load time.
"""

import sys
from contextlib import ExitStack
import numpy as np
import ml_dtypes

sys.path.insert(0, "/opt/trn_rl_repo")

import concourse.bass as bass
import concourse.mybir as mybir
import concourse.tile as tile
from concourse import bacc
from concourse.bass_utils import run_bass_kernel_spmd
from concourse.masks import make_identity

L, H, E, T, V = 4, 12, 768, 1024, 50257
B = 2
NC = 8
TS = (B * T) // NC          # 256 tokens per core
VS = 6284                   # vocab slice per core (padded V = 50272)
VPAD = VS * NC
HD = 64
EPS = 1e-5
SCALE = float(1.0 / np.sqrt(np.float32(E)))
F32 = mybir.dt.float32
F32R = mybir.dt.float32r
BF16 = mybir.dt.bfloat16
BF = ml_dtypes.bfloat16

_CACHE = {}


def _build_program():
    nc = bacc.Bacc("TRN2", target_bir_lowering=False, debug=False, num_devices=NC)

    # ---- I/O -------------------------------------------------------------
    x0s = nc.dram_tensor("x0s", [TS, E], F32, kind="ExternalInput")
    wq = nc.dram_tensor("wq", [L, E, E], BF16, kind="ExternalInput")
    bq = nc.dram_tensor("bq", [L, H, HD], F32, kind="ExternalInput")
    wk = nc.dram_tensor("wk", [L, E, E], BF16, kind="ExternalInput")
    bk = nc.dram_tensor("bk", [L, 6, 128], F32, kind="ExternalInput")
    wv = nc.dram_tensor("wv", [L, E, E], BF16, kind="ExternalInput")
    bv = nc.dram_tensor("bv", [L, E], BF16, kind="ExternalInput")
    watp = nc.dram_tensor("watp", [L, H, HD, E], BF16, kind="ExternalInput")
    atpb = nc.dram_tensor("atpb", [L, E], BF16, kind="ExternalInput")
    fcw = nc.dram_tensor("fcw", [L, 6, 128, 4 * E], BF16, kind="ExternalInput")
    fcb = nc.dram_tensor("fcb", [L, 24, 128], F32, kind="ExternalInput")
    prw = nc.dram_tensor("prw", [L, 4 * E, E], BF16, kind="ExternalInput")
    prb = nc.dram_tensor("prb", [L, E], BF16, kind="ExternalInput")
    ln1g = nc.dram_tensor("ln1g", [L, E], BF16, kind="ExternalInput")
    ln1b = nc.dram_tensor("ln1b", [L, E], BF16, kind="ExternalInput")
    ln2g = nc.dram_tensor("ln2g", [L, E], BF16, kind="ExternalInput")
    ln2b = nc.dram_tensor("ln2b", [L, E], BF16, kind="ExternalInput")
    lnfg = nc.dram_tensor("lnfg", [1, E], BF16, kind="ExternalInput")
    lnfb = nc.dram_tensor("lnfb", [1, E], BF16, kind="ExternalInput")
    mask = nc.dram_tensor("mask", [8, 128, TS], BF16, kind="ExternalInput")
    wteT = nc.dram_tensor("wteT", [E, VS], BF16, kind="ExternalInput")
    logits = nc.dram_tensor("logits", [B * T, VS], F32, kind="ExternalOutput")

    g_all = [list(range(NC))]
    g_batch = [[0, 1, 2, 3], [4, 5, 6, 7]]

    def bcast_row(pool, src_ap, n, dtype, w, name="bc"):
        """Replicate a [w] DRAM row across n partitions via broadcast DMA."""
        t = pool.tile([n, w], dtype, tag=name, name=name)
        in_ap = bass.AP(
            tensor=src_ap.tensor,
            offset=src_ap.offset,
            ap=[[0, n]] + [list(p) for p in src_ap.ap],
        )
        nc.sync.dma_start(out=t[:], in_=in_ap)
        return t

    with tile.TileContext(nc) as tc, ExitStack() as es:
        const = es.enter_context(tc.tile_pool(name="const", bufs=1))
        xp = es.enter_context(tc.tile_pool(name="xp", bufs=1))
        lnrow = es.enter_context(tc.tile_pool(name="lnrow", bufs=2))
        stat = es.enter_context(tc.tile_pool(name="stat", bufs=4))
        hpool = es.enter_context(tc.tile_pool(name="hpool", bufs=2))
        hTp = es.enter_context(tc.tile_pool(name="hTp", bufs=1))
        dram = es.enter_context(tc.tile_pool(name="dram", bufs=1, space="DRAM"))
        psT = es.enter_context(tc.tile_pool(name="psT", bufs=2, space="PSUM"))

        ident_f = const.tile([128, 128], F32, name="ident_f")
        make_identity(nc, ident_f)
        ident = const.tile([128, 128], F32R, name="ident")
        nc.vector.tensor_copy(ident[:], ident_f[:])
        eps_sb = const.tile([128, 1], F32)
        nc.vector.memset(eps_sb, EPS)
        ones_f = const.tile([128, H * 65], F32, name="ones_f")
        nc.vector.memset(ones_f, 1.0)
        ones780 = const.tile([128, H * 65], BF16, name="ones780")
        nc.vector.tensor_copy(ones780[:], ones_f[:])
        mask_sb = [const.tile([128, TS], BF16, name=f"mask{j}") for j in range(8)]
        for j in range(8):
            nc.sync.dma_start(out=mask_sb[j][:], in_=mask[j])

        # persistent residual stream [256, 768] as two [128, 768] f32 tiles
        x_sb = [xp.tile([128, E], F32, tag=f"x{t}", name=f"x{t}") for t in range(2)]
        for t in range(2):
            nc.sync.dma_start(out=x_sb[t][:], in_=x0s[t * 128:(t + 1) * 128, :])

        # DRAM bounce buffers for collectives
        kT_in = dram.tile([E, TS], BF16)
        kT_ag = dram.tile([4 * E, TS], BF16)
        v_in = dram.tile([TS, H * 65], BF16)
        v_ag = dram.tile([4 * TS, H * 65], BF16)
        xf_in = dram.tile([E, TS], BF16)
        xf_ag = dram.tile([NC * E, TS], BF16)

        def layernorm_t(x_ap, g_bc, b_bc, out_tile):
            """LN over free dim (768) of [128, 768] f32 tile."""
            stats = stat.tile([128, 3, 6], F32, tag="bn_stats", name="bn_stats_t")
            xr = x_ap.rearrange("p (s d) -> p s d", s=3)
            for s in range(3):
                nc.vector.bn_stats(out=stats[:, s, :], in_=xr[:, s, :])
            mv = stat.tile([128, 2], F32, tag="bn_aggr", name="bn_aggr_t")
            nc.vector.bn_aggr(out=mv[:], in_=stats[:])
            rstd = stat.tile([128, 1], F32, tag="rstd", name="rstd_t")
            nc.scalar.activation(out=rstd[:], in_=mv[:, 1:2],
                                 func=mybir.ActivationFunctionType.Sqrt,
                                 bias=eps_sb[:], scale=1.0)
            nc.vector.reciprocal(out=rstd[:], in_=rstd[:])
            nc.vector.tensor_scalar(out=out_tile[:], in0=x_ap,
                                    scalar1=mv[:, 0:1], scalar2=rstd[:],
                                    op0=mybir.AluOpType.subtract,
                                    op1=mybir.AluOpType.mult)
            nc.vector.tensor_mul(out=out_tile[:], in0=out_tile[:], in1=g_bc[:])
            nc.vector.tensor_add(out=out_tile[:], in0=out_tile[:], in1=b_bc[:])

        def transpose_to(hsrc, dst_tiles, dst_col):
            """hsrc [128,768] f32r -> bf16 dst_tiles[k][:, dst_col:dst_col+128]."""
            for k in range(6):
                pt = psT.tile([128, 128], F32R, tag="tr", name="tr")
                nc.tensor.transpose(pt[:], hsrc[:, k * 128:(k + 1) * 128], ident[:])
                dst = dst_tiles[k][:, dst_col:dst_col + 128]
                if k % 2 == 0:
                    nc.vector.tensor_copy(dst, pt[:].bitcast(F32))
                else:
                    nc.scalar.activation(out=dst, in_=pt[:].bitcast(F32),
                                         func=mybir.ActivationFunctionType.Copy)

        def scope(name):
            sid, _ = nc.enter_named_scope(name, False)
            _SCOPES.append((name, sid))

        def unscope():
            name, sid = _SCOPES.pop()
            nc.leave_named_scope(name, sid, False)

        _SCOPES = []

        es_l = es.enter_context(ExitStack())
        wqp = es_l.enter_context(tc.tile_pool(name="wqp", bufs=1))
        wkp = es_l.enter_context(tc.tile_pool(name="wkp", bufs=1))
        wvp = es_l.enter_context(tc.tile_pool(name="wvp", bufs=1))
        watpp = es_l.enter_context(tc.tile_pool(name="watpp", bufs=1))
        bias_p = es_l.enter_context(tc.tile_pool(name="bias_p", bufs=2))
        qTp = es_l.enter_context(tc.tile_pool(name="qTp", bufs=1))
        kvsp = es_l.enter_context(tc.tile_pool(name="kvsp", bufs=2))
        kTp = es_l.enter_context(tc.tile_pool(name="kTp", bufs=1))
        vp = es_l.enter_context(tc.tile_pool(name="vp", bufs=1))
        ep = es_l.enter_context(tc.tile_pool(name="ep", bufs=3))
        yp = es_l.enter_context(tc.tile_pool(name="yp", bufs=1))
        sm = es_l.enter_context(tc.tile_pool(name="sm", bufs=2))
        fcwp = es_l.enter_context(tc.tile_pool(name="fcwp", bufs=1))
        mTp = es_l.enter_context(tc.tile_pool(name="mTp", bufs=1))
        prwp = es_l.enter_context(tc.tile_pool(name="prwp", bufs=3))

        for layer in range(L):
            scope(f"L{layer}_qkv")
            ln1g_bc = bcast_row(lnrow, ln1g[layer], 128, BF16, E, "ln_g")
            ln1b_bc = bcast_row(lnrow, ln1b[layer], 128, BF16, E, "ln_b")

            # ---- LN1 + transpose -> hT [768, 256] bf16 ----
            hT = [hTp.tile([128, TS], BF16, tag=f"hT{k}", name=f"hT{k}") for k in range(6)]
            for t in range(2):
                h_t = hpool.tile([128, E], F32R, tag="h", name="h")
                layernorm_t(x_sb[t][:], ln1g_bc, ln1b_bc, h_t)
                transpose_to(h_t, hT, t * 128)

            # ---- K (dim-major, per-partition bias) ----
            wk_sb = [wkp.tile([128, E], BF16, tag=f"wk{k}", name=f"wk{k}")
                     for k in range(6)]
            for k in range(6):
                nc.sync.dma_start(out=wk_sb[k][:],
                                  in_=wk[layer, k * 128:(k + 1) * 128, :])
            bk_sb = bias_p.tile([128, 6], F32, tag="bk", name="bk")
            nc.sync.dma_start(out=bk_sb[:], in_=bk[layer].transpose([1, 0]))
            es_a = ExitStack()
            psK = es_a.enter_context(tc.tile_pool(name="psK", bufs=2, space="PSUM"))
            psV = es_a.enter_context(tc.tile_pool(name="psV", bufs=1, space="PSUM"))
            for m in range(6):
                ps = psK.tile([128, TS], F32, tag="kps", name="k_ps")
                for k in range(6):
                    nc.tensor.matmul(ps[:], wk_sb[k][:, m * 128:(m + 1) * 128],
                                     hT[k][:], start=(k == 0), stop=(k == 5))
                kt = kvsp.tile([128, TS], BF16, tag="kloc", name="kloc")
                nc.vector.tensor_scalar_add(out=kt[:], in0=ps[:],
                                            scalar1=bk_sb[:, m:m + 1])
                nc.sync.dma_start(out=kT_in[m * 128:(m + 1) * 128, :], in_=kt[:])
            nc.gpsimd.collective_compute(
                "AllGather", mybir.AluOpType.bypass,
                replica_groups=g_batch,
                ins=[kT_in.opt()],
                outs=[kT_ag.opt()],
            )

            # ---- V (token-major, ones column pre-packed) ----
            wv_sb = [wvp.tile([128, E], BF16, tag=f"wv{k}", name=f"wv{k}")
                     for k in range(6)]
            for k in range(6):
                nc.sync.dma_start(out=wv_sb[k][:],
                                  in_=wv[layer, k * 128:(k + 1) * 128, :])
            bv_bc = bcast_row(lnrow, bv[layer], 128, BF16, E, "bv")
            for t in range(2):
                ps = psV.tile([128, E], F32, tag="vps", name="v_ps")
                for n0, n1 in ((0, 512), (512, 768)):
                    for k in range(6):
                        nc.tensor.matmul(ps[:, n0:n1],
                                         hT[k][:, t * 128:(t + 1) * 128],
                                         wv_sb[k][:, n0:n1],
                                         start=(k == 0), stop=(k == 5))
                vt = kvsp.tile([128, H * 65], BF16, tag="vloc", name="vloc")
                nc.vector.tensor_copy(vt[:], ones780[:])
                for h in range(H):
                    nc.vector.tensor_add(out=vt[:, 65 * h:65 * h + 64],
                                         in0=ps[:, 64 * h:64 * h + 64],
                                         in1=bv_bc[:, 64 * h:64 * h + 64])
                nc.sync.dma_start(out=v_in[t * 128:(t + 1) * 128, :], in_=vt[:])
            nc.gpsimd.collective_compute(
                "AllGather", mybir.AluOpType.bypass,
                replica_groups=g_batch,
                ins=[v_in.opt()],
                outs=[v_ag.opt()],
            )

            # ---- Q per head (overlaps the AllGathers) ----
            wq_sb = [wqp.tile([128, E], BF16, tag=f"wq{k}", name=f"wq{k}")
                     for k in range(6)]
            for k in range(6):
                nc.sync.dma_start(out=wq_sb[k][:],
                                  in_=wq[layer, k * 128:(k + 1) * 128, :])
            bq_sb = bias_p.tile([64, H], F32, tag="bq", name="bq")
            nc.sync.dma_start(out=bq_sb[:], in_=bq[layer].transpose([1, 0]))
            psQ = es_a.enter_context(tc.tile_pool(name="psQ", bufs=2, space="PSUM"))
            qT = [qTp.tile([64, TS], BF16, tag=f"qT{h}", name=f"qT{h}") for h in range(H)]
            for h in range(H):
                ps = psQ.tile([64, TS], F32, tag="q", name="q_ps")
                for k in range(6):
                    nc.tensor.matmul(ps[:], wq_sb[k][:, h * 64:(h + 1) * 64],
                                     hT[k][:], start=(k == 0), stop=(k == 5))
                nc.vector.tensor_scalar_add(out=qT[h][:], in0=ps[:],
                                            scalar1=bq_sb[:, h:h + 1])
            es_a.close()

            unscope()
            scope(f"L{layer}_attn")
            # ---- load gathered K (per-head, base-0) and V (plain) ----
            kTh = [kTp.tile([64, T], BF16, tag=f"kTh{h}", name=f"kTh{h}")
                   for h in range(H)]
            kT_view = kT_ag[:].rearrange("(r m q d) c -> r m q d c", r=4, m=6, q=2)
            for h in range(H):
                nc.sync.dma_start(
                    out=kTh[h][:].rearrange("p (r c) -> p r c", r=4),
                    in_=kT_view[:, h // 2, h % 2].transpose([1, 0, 2]))
            v_sb = [vp.tile([128, H * 65], BF16, tag=f"v{j}", name=f"v{j}")
                    for j in range(8)]
            for j in range(8):
                nc.sync.dma_start(out=v_sb[j][:],
                                  in_=v_ag[j * 128:(j + 1) * 128, :])

            # ---- attention: 12 heads, 256 queries vs 1024 masked keys ----
            es_b = ExitStack()
            psS = es_b.enter_context(tc.tile_pool(name="psS", bufs=2, space="PSUM"))
            psY = es_b.enter_context(tc.tile_pool(name="psY", bufs=2, space="PSUM"))
            yT = [yp.tile([64, TS], BF16, tag=f"yT{h}", name=f"yT{h}") for h in range(H)]
            for h in range(H):
                yps = psY.tile([65, TS], F32, tag="y", name="y_ps")
                for j in range(8):
                    pss = psS.tile([128, TS], F32, tag="s", name="s_ps")
                    nc.tensor.matmul(pss[:], kTh[h][:, j * 128:(j + 1) * 128],
                                     qT[h][:], start=True, stop=True)
                    e_sb = ep.tile([128, TS], BF16, tag="e", name="e")
                    nc.scalar.activation(out=e_sb[:], in_=pss[:],
                                         func=mybir.ActivationFunctionType.Exp,
                                         scale=SCALE)
                    nc.vector.tensor_mul(out=e_sb[:], in0=e_sb[:], in1=mask_sb[j][:])
                    nc.tensor.matmul(yps[:], v_sb[j][:, 65 * h:65 * h + 65],
                                     e_sb[:], start=(j == 0), stop=(j == 7))
                recip = sm.tile([1, TS], F32, tag="recip", name="recip")
                nc.vector.reciprocal(out=recip[:], in_=yps[64:65, :])
                recip_bc = sm.tile([64, TS], F32, tag="recip_bc", name="recip_bc")
                nc.gpsimd.partition_broadcast(recip_bc[:], recip[:])
                nc.vector.tensor_mul(out=yT[h][:], in0=yps[0:64, :], in1=recip_bc[:])
            es_b.close()

            # ---- out-proj (token-local) + residual ----
            es_c = ExitStack()
            psO = es_c.enter_context(tc.tile_pool(name="psO", bufs=2, space="PSUM"))
            watp_sb = [watpp.tile([64, E], BF16, tag=f"watp{h}", name=f"watp{h}")
                       for h in range(H)]
            for h in range(H):
                nc.sync.dma_start(out=watp_sb[h][:], in_=watp[layer, h])
            atpb_bc = bcast_row(lnrow, atpb[layer], 128, BF16, E, "atpb")
            for t in range(2):
                ps = psO.tile([128, E], F32, tag="o", name="o_ps")
                for n0, n1 in ((0, 512), (512, 768)):
                    for h in range(H):
                        nc.tensor.matmul(ps[:, n0:n1],
                                         yT[h][:, t * 128:(t + 1) * 128],
                                         watp_sb[h][:, n0:n1],
                                         start=(h == 0), stop=(h == H - 1))
                nc.vector.tensor_add(out=x_sb[t][:], in0=x_sb[t][:], in1=ps[:])
                nc.vector.tensor_add(out=x_sb[t][:], in0=x_sb[t][:], in1=atpb_bc[:])
            es_c.close()

            unscope()
            scope(f"L{layer}_mlp")
            # ---- LN2 + transpose ----
            ln2g_bc = bcast_row(lnrow, ln2g[layer], 128, BF16, E, "ln_g")
            ln2b_bc = bcast_row(lnrow, ln2b[layer], 128, BF16, E, "ln_b")
            h2T = [hTp.tile([128, TS], BF16, tag=f"hT{k}", name=f"hT{k}") for k in range(6)]
            for t in range(2):
                h_t = hpool.tile([128, E], F32R, tag="h", name="h")
                layernorm_t(x_sb[t][:], ln2g_bc, ln2b_bc, h_t)
                transpose_to(h_t, h2T, t * 128)

            # ---- MLP fc: mT[m] = gelu(fc_w.T @ h2T + fc_b) ----
            es_d = ExitStack()
            psM = es_d.enter_context(tc.tile_pool(name="psM", bufs=2, space="PSUM"))
            psP = es_d.enter_context(tc.tile_pool(name="psP", bufs=1, space="PSUM"))
            fcb_sb = bias_p.tile([128, 24], F32, tag="fcb", name="fcb")
            nc.sync.dma_start(out=fcb_sb[:], in_=fcb[layer].transpose([1, 0]))
            fck = [fcwp.tile([128, 4 * E], BF16, tag=f"fck{k}", name=f"fck{k}")
                   for k in range(6)]
            for k in range(6):
                nc.sync.dma_start(out=fck[k][:], in_=fcw[layer, k])
            mT = [mTp.tile([128, TS], BF16, tag=f"mT{m}", name=f"mT{m}") for m in range(24)]
            for m in range(24):
                ps = psM.tile([128, TS], F32, tag="m", name="m_ps")
                for k in range(6):
                    nc.tensor.matmul(ps[:], fck[k][:, m * 128:(m + 1) * 128],
                                     h2T[k][:], start=(k == 0), stop=(k == 5))
                nc.scalar.activation(out=mT[m][:], in_=ps[:],
                                     func=mybir.ActivationFunctionType.Gelu_apprx_tanh,
                                     bias=fcb_sb[:, m:m + 1])

            # ---- MLP pr + residual ----
            prb_bc = bcast_row(lnrow, prb[layer], 128, BF16, E, "prb")
            ps2 = [psP.tile([128, E], F32, tag=f"p{t}", name=f"p{t}") for t in range(2)]
            for kk in range(24):
                prw_sb = prwp.tile([128, E], BF16, tag="prw", name="prw")
                nc.sync.dma_start(out=prw_sb[:],
                                  in_=prw[layer, kk * 128:(kk + 1) * 128, :])
                for t in range(2):
                    for n0, n1 in ((0, 512), (512, 768)):
                        nc.tensor.matmul(ps2[t][:, n0:n1],
                                         mT[kk][:, t * 128:(t + 1) * 128],
                                         prw_sb[:, n0:n1],
                                         start=(kk == 0), stop=(kk == 23))
            for t in range(2):
                nc.vector.tensor_add(out=x_sb[t][:], in0=x_sb[t][:], in1=ps2[t][:])
                nc.vector.tensor_add(out=x_sb[t][:], in0=x_sb[t][:], in1=prb_bc[:])
            es_d.close()
            unscope()

        # ---- final LN + AllGather(all 8) + lm_head ----
        scope("lnf_ag")
        lnfg_bc = bcast_row(lnrow, lnfg[0], 128, BF16, E, "ln_g")
        lnfb_bc = bcast_row(lnrow, lnfb[0], 128, BF16, E, "ln_b")
        xfT = [hTp.tile([128, TS], BF16, tag=f"hT{k}", name=f"hT{k}") for k in range(6)]
        for t in range(2):
            h_t = hpool.tile([128, E], F32R, tag="h", name="h")
            layernorm_t(x_sb[t][:], lnfg_bc, lnfb_bc, h_t)
            transpose_to(h_t, xfT, t * 128)
        for k in range(6):
            nc.sync.dma_start(out=xf_in[k * 128:(k + 1) * 128, :], in_=xfT[k][:])
        nc.gpsimd.collective_compute(
            "AllGather", mybir.AluOpType.bypass,
            replica_groups=g_all,
            ins=[xf_in.opt()],
            outs=[xf_ag.opt()],
        )
        es_l.close()
        es_h = es.enter_context(ExitStack())
        xfp = es_h.enter_context(tc.tile_pool(name="xfp", bufs=1))
        wtep = es_h.enter_context(tc.tile_pool(name="wtep", bufs=3))
        lop = es_h.enter_context(tc.tile_pool(name="lop", bufs=4))
        psL = es_h.enter_context(tc.tile_pool(name="psL", bufs=4, space="PSUM"))

        xf_sb = [xfp.tile([128, TS], BF16, tag=f"xf{i}", name=f"xf{i}") for i in range(48)]
        for i in range(48):
            nc.sync.dma_start(out=xf_sb[i][:], in_=xf_ag[i * 128:(i + 1) * 128, :])
        unscope()

        scope("lmhead")
        nch = (VS + 511) // 512
        for n in range(nch):
            n0 = n * 512
            nw = min(512, VS - n0)
            wte_sb = [wtep.tile([128, 512], BF16, tag=f"wte{k}", name=f"wte{k}")
                      for k in range(6)]
            for k in range(6):
                nc.sync.dma_start(out=wte_sb[k][:, 0:nw],
                                  in_=wteT[k * 128:(k + 1) * 128, n0:n0 + nw])
            for t in range(16):
                r, half = t // 2, t % 2
                ps = psL.tile([128, 512], F32, tag="l", name="l_ps")
                for k in range(6):
                    nc.tensor.matmul(ps[:, 0:nw],
                                     xf_sb[r * 6 + k][:, half * 128:(half + 1) * 128],
                                     wte_sb[k][:, 0:nw],
                                     start=(k == 0), stop=(k == 5))
                lo = lop.tile([128, 512], F32, tag="lo", name="lo")
                if t % 2 == 0:
                    nc.vector.tensor_copy(lo[:, 0:nw], ps[:, 0:nw])
                else:
                    nc.scalar.activation(out=lo[:, 0:nw], in_=ps[:, 0:nw],
                                         func=mybir.ActivationFunctionType.Copy)
                nc.sync.dma_start(out=logits[t * 128:(t + 1) * 128, n0:n0 + nw],
                                  in_=lo[:, 0:nw])
        unscope()

    nc.compile()
    return nc


def _prep_inputs(idx, wte, wpe, ln1_w, ln1_b, attn_w, attn_b, atp_w, atp_b,
                 ln2_w, ln2_b, fc_w, fc_b, pr_w, pr_b, lnf_w, lnf_b):
    idx = np.asarray(idx)
    f = lambda a: np.ascontiguousarray(np.asarray(a), dtype=np.float32)
    bf = lambda a: np.ascontiguousarray(np.asarray(a, dtype=np.float32).astype(BF))
    wte32, wpe32 = f(wte), f(wpe)
    x0 = wte32[idx.reshape(-1)] + np.tile(wpe32[:T], (B, 1))  # [2048, 768]
    wte_pad = np.zeros((VPAD, E), np.float32)
    wte_pad[:V] = wte32
    wteT_full = np.ascontiguousarray(wte_pad.T).astype(BF)  # [768, VPAD]

    attn_w, attn_b = f(attn_w), f(attn_b)
    common = {
        "wq": bf(attn_w[:, :, 0:E]),
        "bq": np.ascontiguousarray(attn_b[:, 0:E].reshape(L, H, HD)),
        "wk": bf(attn_w[:, :, E:2 * E]),
        "bk": np.ascontiguousarray(attn_b[:, E:2 * E].reshape(L, 6, 128)),
        "wv": bf(attn_w[:, :, 2 * E:3 * E]),
        "bv": bf(attn_b[:, 2 * E:3 * E]),
        "watp": bf(np.asarray(atp_w).reshape(L, H, HD, E)),
        "atpb": bf(atp_b),
        "fcw": bf(np.asarray(fc_w).reshape(L, 6, 128, 4 * E)),
        "fcb": np.ascontiguousarray(f(fc_b).reshape(L, 24, 128)),
        "prw": bf(pr_w), "prb": bf(pr_b),
        "ln1g": bf(ln1_w), "ln1b": bf(ln1_b),
        "ln2g": bf(ln2_w), "ln2b": bf(ln2_b),
        "lnfg": bf(lnf_w).reshape(1, E), "lnfb": bf(lnf_b).reshape(1, E),
    }
    in_maps = []
    kidx = np.arange(128)
    qidx = np.arange(TS)
    for c in range(NC):
        r = c % 4
        m = np.zeros((8, 128, TS), np.float32)
        for j in range(8):
            m[j] = ((128 * j + kidx)[:, None] <= (TS * r + qidx)[None, :])
        in_maps.append({
            **common,
            "x0s": np.ascontiguousarray(x0[c * TS:(c + 1) * TS]),
            "mask": m.astype(BF),
            "wteT": np.ascontiguousarray(wteT_full[:, c * VS:(c + 1) * VS]),
        })
    return in_maps


def kernel(trace=False, **inputs):
    if "nc" not in _CACHE:
        _CACHE["nc"] = _build_program()
    nc = _CACHE["nc"]
    in_maps = _prep_inputs(**inputs)
    res = run_bass_kernel_spmd(nc, in_maps, core_ids=list(range(NC)), trace=trace)
    _CACHE["last_result"] = res
    logits = np.concatenate([res.results[c]["logits"] for c in range(NC)], axis=1)
    return logits[:, :V].reshape(B, T, V).astype(np.float32)


# revision 23
# speedup vs baseline: 1.4033x; 1.0396x over previous
"""GPT (4-layer, E=768, H=12, T=1024, B=2, V=50257) forward on 8 trn2 cores.

Sharding (v3):
  - Token-parallel residual: core c owns tokens [r*256,(r+1)*256) of batch
    c//4 (r = c%4).  LN / QKV / out-proj / MLP fully token-local.
  - Attention: per layer two AllGathers per batch group of 4 — K (dim-major
    [768,256] bf16) and V (token-major [256,780] bf16, softmax ones column
    pre-packed).  Each core computes all 12 heads for its own 256 queries
    against the full 1024 keys with a per-core causal mask (uniform SPMD
    program; masking by data, not control flow).  No ReduceScatter.
  - lm_head vocab-sharded: final AllGather of lnf(x) (transposed, bf16),
    each core computes a [2048, 6284] logit slice (V padded to 50272).
  - All matmul operands bf16 (fp32 PSUM accumulation); weights shipped bf16
    to halve HBM traffic and PE weight-load time.
"""

import sys
from contextlib import ExitStack
import numpy as np
import ml_dtypes

sys.path.insert(0, "/opt/trn_rl_repo")

import concourse.bass as bass
import concourse.mybir as mybir
import concourse.tile as tile
from concourse import bacc
from concourse.bass_utils import run_bass_kernel_spmd
from concourse.masks import make_identity

L, H, E, T, V = 4, 12, 768, 1024, 50257
B = 2
NC = 8
TS = (B * T) // NC          # 256 tokens per core
VS = 6284                   # vocab slice per core (padded V = 50272)
VPAD = VS * NC
HD = 64
EPS = 1e-5
SCALE = float(1.0 / np.sqrt(np.float32(E)))
F32 = mybir.dt.float32
F32R = mybir.dt.float32r
BF16 = mybir.dt.bfloat16
BF = ml_dtypes.bfloat16

_CACHE = {}


def _build_program():
    nc = bacc.Bacc("TRN2", target_bir_lowering=False, debug=False, num_devices=NC)

    # ---- I/O -------------------------------------------------------------
    x0s = nc.dram_tensor("x0s", [TS, E], F32, kind="ExternalInput")
    wq = nc.dram_tensor("wq", [L, E, E], BF16, kind="ExternalInput")
    bq = nc.dram_tensor("bq", [L, H, HD], F32, kind="ExternalInput")
    wk = nc.dram_tensor("wk", [L, E, E], BF16, kind="ExternalInput")
    bk = nc.dram_tensor("bk", [L, 6, 128], F32, kind="ExternalInput")
    wv = nc.dram_tensor("wv", [L, E, E], BF16, kind="ExternalInput")
    bv = nc.dram_tensor("bv", [L, E], BF16, kind="ExternalInput")
    watp = nc.dram_tensor("watp", [L, H, HD, E], BF16, kind="ExternalInput")
    atpb = nc.dram_tensor("atpb", [L, E], BF16, kind="ExternalInput")
    fcw = nc.dram_tensor("fcw", [L, 6, 128, 4 * E], BF16, kind="ExternalInput")
    fcb = nc.dram_tensor("fcb", [L, 24, 128], F32, kind="ExternalInput")
    prw = nc.dram_tensor("prw", [L, 4 * E, E], BF16, kind="ExternalInput")
    prb = nc.dram_tensor("prb", [L, E], BF16, kind="ExternalInput")
    ln1g = nc.dram_tensor("ln1g", [L, E], BF16, kind="ExternalInput")
    ln1b = nc.dram_tensor("ln1b", [L, E], BF16, kind="ExternalInput")
    ln2g = nc.dram_tensor("ln2g", [L, E], BF16, kind="ExternalInput")
    ln2b = nc.dram_tensor("ln2b", [L, E], BF16, kind="ExternalInput")
    lnfg = nc.dram_tensor("lnfg", [1, E], BF16, kind="ExternalInput")
    lnfb = nc.dram_tensor("lnfb", [1, E], BF16, kind="ExternalInput")
    mask = nc.dram_tensor("mask", [8, 128, 2 * TS], BF16, kind="ExternalInput")
    wteT = nc.dram_tensor("wteT", [E, VS], BF16, kind="ExternalInput")
    logits = nc.dram_tensor("logits", [B * T, VS], F32, kind="ExternalOutput")

    g_all = [list(range(NC))]
    g_batch = [[0, 1, 2, 3], [4, 5, 6, 7]]

    def bcast_row(pool, src_ap, n, dtype, w, name="bc"):
        """Replicate a [w] DRAM row across n partitions via broadcast DMA."""
        t = pool.tile([n, w], dtype, tag=name, name=name)
        in_ap = bass.AP(
            tensor=src_ap.tensor,
            offset=src_ap.offset,
            ap=[[0, n]] + [list(p) for p in src_ap.ap],
        )
        nc.sync.dma_start(out=t[:], in_=in_ap)
        return t

    with tile.TileContext(nc) as tc, ExitStack() as es:
        const = es.enter_context(tc.tile_pool(name="const", bufs=1))
        xp = es.enter_context(tc.tile_pool(name="xp", bufs=1))
        lnrow = es.enter_context(tc.tile_pool(name="lnrow", bufs=2))
        stat = es.enter_context(tc.tile_pool(name="stat", bufs=4))
        hpool = es.enter_context(tc.tile_pool(name="hpool", bufs=2))
        hTp = es.enter_context(tc.tile_pool(name="hTp", bufs=1))
        dram = es.enter_context(tc.tile_pool(name="dram", bufs=1, space="DRAM"))
        psT = es.enter_context(tc.tile_pool(name="psT", bufs=2, space="PSUM"))

        ident_f = const.tile([128, 128], F32, name="ident_f")
        make_identity(nc, ident_f)
        ident = const.tile([128, 128], F32R, name="ident")
        nc.vector.tensor_copy(ident[:], ident_f[:])
        eps_sb = const.tile([128, 1], F32)
        nc.vector.memset(eps_sb, EPS)
        ones_f = const.tile([128, H * 65], F32, name="ones_f")
        nc.vector.memset(ones_f, 1.0)
        ones780 = const.tile([128, H * 65], BF16, name="ones780")
        nc.vector.tensor_copy(ones780[:], ones_f[:])
        mask_sb = [const.tile([128, 2 * TS], BF16, name=f"mask{j}") for j in range(8)]
        for j in range(8):
            nc.sync.dma_start(out=mask_sb[j][:], in_=mask[j])

        # persistent residual stream [256, 768] as two [128, 768] f32 tiles
        x_sb = [xp.tile([128, E], F32, tag=f"x{t}", name=f"x{t}") for t in range(2)]
        for t in range(2):
            nc.sync.dma_start(out=x_sb[t][:], in_=x0s[t * 128:(t + 1) * 128, :])

        # DRAM bounce buffers for collectives
        kT_in = dram.tile([E, TS], BF16)
        kT_ag = dram.tile([4 * E, TS], BF16)
        v_in = dram.tile([TS, H * 65], BF16)
        v_ag = dram.tile([4 * TS, H * 65], BF16)
        xf_in = dram.tile([E, TS], BF16)
        xf_ag = dram.tile([NC * E, TS], BF16, addr_space="Shared")

        def layernorm_t(x_ap, g_bc, b_bc, out_tile):
            """LN over free dim (768) of [128, 768] f32 tile."""
            stats = stat.tile([128, 3, 6], F32, tag="bn_stats", name="bn_stats_t")
            xr = x_ap.rearrange("p (s d) -> p s d", s=3)
            for s in range(3):
                nc.vector.bn_stats(out=stats[:, s, :], in_=xr[:, s, :])
            mv = stat.tile([128, 2], F32, tag="bn_aggr", name="bn_aggr_t")
            nc.vector.bn_aggr(out=mv[:], in_=stats[:])
            rstd = stat.tile([128, 1], F32, tag="rstd", name="rstd_t")
            nc.scalar.activation(out=rstd[:], in_=mv[:, 1:2],
                                 func=mybir.ActivationFunctionType.Sqrt,
                                 bias=eps_sb[:], scale=1.0)
            nc.vector.reciprocal(out=rstd[:], in_=rstd[:])
            nc.vector.tensor_scalar(out=out_tile[:], in0=x_ap,
                                    scalar1=mv[:, 0:1], scalar2=rstd[:],
                                    op0=mybir.AluOpType.subtract,
                                    op1=mybir.AluOpType.mult)
            nc.vector.tensor_mul(out=out_tile[:], in0=out_tile[:], in1=g_bc[:])
            nc.vector.tensor_add(out=out_tile[:], in0=out_tile[:], in1=b_bc[:])

        def transpose_to(hsrc, dst_tiles, dst_col):
            """hsrc [128,768] f32r -> bf16 dst_tiles[k][:, dst_col:dst_col+128]."""
            for k in range(6):
                pt = psT.tile([128, 128], F32R, tag="tr", name="tr")
                nc.tensor.transpose(pt[:], hsrc[:, k * 128:(k + 1) * 128], ident[:])
                dst = dst_tiles[k][:, dst_col:dst_col + 128]
                if k % 2 == 0:
                    nc.vector.tensor_copy(dst, pt[:].bitcast(F32))
                else:
                    nc.scalar.activation(out=dst, in_=pt[:].bitcast(F32),
                                         func=mybir.ActivationFunctionType.Copy)

        def scope(name):
            sid, _ = nc.enter_named_scope(name, False)
            _SCOPES.append((name, sid))

        def unscope():
            name, sid = _SCOPES.pop()
            nc.leave_named_scope(name, sid, False)

        _SCOPES = []

        es_l = es.enter_context(ExitStack())
        wqp = es_l.enter_context(tc.tile_pool(name="wqp", bufs=1))
        wkp = es_l.enter_context(tc.tile_pool(name="wkp", bufs=1))
        wvp = es_l.enter_context(tc.tile_pool(name="wvp", bufs=1))
        watpp = es_l.enter_context(tc.tile_pool(name="watpp", bufs=1))
        bias_p = es_l.enter_context(tc.tile_pool(name="bias_p", bufs=2))
        qTp = es_l.enter_context(tc.tile_pool(name="qTp", bufs=1))
        kvsp = es_l.enter_context(tc.tile_pool(name="kvsp", bufs=2))
        kTp = es_l.enter_context(tc.tile_pool(name="kTp", bufs=1))
        vp = es_l.enter_context(tc.tile_pool(name="vp", bufs=1))
        ep = es_l.enter_context(tc.tile_pool(name="ep", bufs=3))
        yp = es_l.enter_context(tc.tile_pool(name="yp", bufs=1))
        sm = es_l.enter_context(tc.tile_pool(name="sm", bufs=2))
        fcwp = es_l.enter_context(tc.tile_pool(name="fcwp", bufs=1))
        mTp = es_l.enter_context(tc.tile_pool(name="mTp", bufs=1))
        prwp = es_l.enter_context(tc.tile_pool(name="prwp", bufs=3))

        for layer in range(L):
            scope(f"L{layer}_qkv")
            ln1g_bc = bcast_row(lnrow, ln1g[layer], 128, BF16, E, "ln_g")
            ln1b_bc = bcast_row(lnrow, ln1b[layer], 128, BF16, E, "ln_b")

            # ---- LN1 + transpose -> hT [768, 256] bf16 ----
            hT = [hTp.tile([128, TS], BF16, tag=f"hT{k}", name=f"hT{k}") for k in range(6)]
            for t in range(2):
                h_t = hpool.tile([128, E], F32R, tag="h", name="h")
                layernorm_t(x_sb[t][:], ln1g_bc, ln1b_bc, h_t)
                transpose_to(h_t, hT, t * 128)

            # ---- K (dim-major, per-partition bias) ----
            wk_sb = [wkp.tile([128, E], BF16, tag=f"wk{k}", name=f"wk{k}")
                     for k in range(6)]
            for k in range(6):
                nc.sync.dma_start(out=wk_sb[k][:],
                                  in_=wk[layer, k * 128:(k + 1) * 128, :])
            bk_sb = bias_p.tile([128, 6], F32, tag="bk", name="bk")
            nc.sync.dma_start(out=bk_sb[:], in_=bk[layer].transpose([1, 0]))
            es_a = ExitStack()
            psK = es_a.enter_context(tc.tile_pool(name="psK", bufs=2, space="PSUM"))
            psV = es_a.enter_context(tc.tile_pool(name="psV", bufs=1, space="PSUM"))
            for m in range(6):
                ps = psK.tile([128, TS], F32, tag="kps", name="k_ps")
                for k in range(6):
                    nc.tensor.matmul(ps[:], wk_sb[k][:, m * 128:(m + 1) * 128],
                                     hT[k][:], start=(k == 0), stop=(k == 5))
                kt = kvsp.tile([128, TS], BF16, tag="kloc", name="kloc")
                nc.vector.tensor_scalar_add(out=kt[:], in0=ps[:],
                                            scalar1=bk_sb[:, m:m + 1])
                nc.sync.dma_start(out=kT_in[m * 128:(m + 1) * 128, :], in_=kt[:])
            nc.gpsimd.collective_compute(
                "AllGather", mybir.AluOpType.bypass,
                replica_groups=g_batch,
                ins=[kT_in.opt()],
                outs=[kT_ag.opt()],
            )

            # ---- V (token-major, ones column pre-packed) ----
            wv_sb = [wvp.tile([128, E], BF16, tag=f"wv{k}", name=f"wv{k}")
                     for k in range(6)]
            for k in range(6):
                nc.sync.dma_start(out=wv_sb[k][:],
                                  in_=wv[layer, k * 128:(k + 1) * 128, :])
            bv_bc = bcast_row(lnrow, bv[layer], 128, BF16, E, "bv")
            for t in range(2):
                ps = psV.tile([128, E], F32, tag="vps", name="v_ps")
                for n0, n1 in ((0, 512), (512, 768)):
                    for k in range(6):
                        nc.tensor.matmul(ps[:, n0:n1],
                                         hT[k][:, t * 128:(t + 1) * 128],
                                         wv_sb[k][:, n0:n1],
                                         start=(k == 0), stop=(k == 5))
                vt = kvsp.tile([128, H * 65], BF16, tag="vloc", name="vloc")
                nc.vector.tensor_copy(vt[:], ones780[:])
                for h in range(H):
                    nc.vector.tensor_add(out=vt[:, 65 * h:65 * h + 64],
                                         in0=ps[:, 64 * h:64 * h + 64],
                                         in1=bv_bc[:, 64 * h:64 * h + 64])
                nc.sync.dma_start(out=v_in[t * 128:(t + 1) * 128, :], in_=vt[:])
            nc.gpsimd.collective_compute(
                "AllGather", mybir.AluOpType.bypass,
                replica_groups=g_batch,
                ins=[v_in.opt()],
                outs=[v_ag.opt()],
            )

            # ---- Q per head (overlaps the AllGathers) ----
            wq_sb = [wqp.tile([128, E], BF16, tag=f"wq{k}", name=f"wq{k}")
                     for k in range(6)]
            for k in range(6):
                nc.sync.dma_start(out=wq_sb[k][:],
                                  in_=wq[layer, k * 128:(k + 1) * 128, :])
            bq_sb = bias_p.tile([64, H], F32, tag="bq", name="bq")
            nc.sync.dma_start(out=bq_sb[:], in_=bq[layer].transpose([1, 0]))
            psQ = es_a.enter_context(tc.tile_pool(name="psQ", bufs=2, space="PSUM"))
            qT = [qTp.tile([64, TS], BF16, tag=f"qT{h}", name=f"qT{h}") for h in range(H)]
            for h in range(H):
                ps = psQ.tile([64, TS], F32, tag="q", name="q_ps")
                for k in range(6):
                    nc.tensor.matmul(ps[:], wq_sb[k][:, h * 64:(h + 1) * 64],
                                     hT[k][:], start=(k == 0), stop=(k == 5))
                nc.vector.tensor_scalar_add(out=qT[h][:], in0=ps[:],
                                            scalar1=bq_sb[:, h:h + 1])
            es_a.close()

            unscope()
            scope(f"L{layer}_attn")
            # ---- load gathered K (per-head, base-0) and V (plain) ----
            kTh = [kTp.tile([64, T], BF16, tag=f"kTh{h}", name=f"kTh{h}")
                   for h in range(H)]
            kT_view = kT_ag[:].rearrange("(r m q d) c -> r m q d c", r=4, m=6, q=2)
            for h in range(H):
                nc.sync.dma_start(
                    out=kTh[h][:].rearrange("p (r c) -> p r c", r=4),
                    in_=kT_view[:, h // 2, h % 2].transpose([1, 0, 2]))
            v_sb = [vp.tile([128, H * 65], BF16, tag=f"v{j}", name=f"v{j}")
                    for j in range(8)]
            for j in range(8):
                nc.sync.dma_start(out=v_sb[j][:],
                                  in_=v_ag[j * 128:(j + 1) * 128, :])

            # ---- attention: 12 heads, 256 queries vs 1024 masked keys ----
            es_b = ExitStack()
            psS = es_b.enter_context(tc.tile_pool(name="psS", bufs=2, space="PSUM"))
            psY = es_b.enter_context(tc.tile_pool(name="psY", bufs=2, space="PSUM"))
            yT = [yp.tile([64, TS], BF16, tag=f"yT{h}", name=f"yT{h}") for h in range(H)]
            for hp in range(6):
                yps = [psY.tile([65, TS], F32, tag=f"y{i}", name=f"y{i}")
                       for i in range(2)]
                for j in range(8):
                    pss = psS.tile([128, 2 * TS], F32, tag="s", name="s_ps")
                    e_sb = ep.tile([128, 2 * TS], BF16, tag="e", name="e")
                    for i in range(2):
                        nc.tensor.matmul(pss[:, i * TS:(i + 1) * TS],
                                         kTh[2 * hp + i][:, j * 128:(j + 1) * 128],
                                         qT[2 * hp + i][:], start=True, stop=True)
                    nc.scalar.activation(out=e_sb[:], in_=pss[:],
                                         func=mybir.ActivationFunctionType.Exp,
                                         scale=SCALE)
                    nc.vector.tensor_mul(out=e_sb[:], in0=e_sb[:], in1=mask_sb[j][:])
                    for i in range(2):
                        h = 2 * hp + i
                        nc.tensor.matmul(yps[i][:], v_sb[j][:, 65 * h:65 * h + 65],
                                         e_sb[:, i * TS:(i + 1) * TS],
                                         start=(j == 0), stop=(j == 7))
                for i in range(2):
                    h = 2 * hp + i
                    recip = sm.tile([1, TS], F32, tag="recip", name="recip")
                    nc.vector.reciprocal(out=recip[:], in_=yps[i][64:65, :])
                    recip_bc = sm.tile([64, TS], F32, tag="recip_bc", name="recip_bc")
                    nc.gpsimd.partition_broadcast(recip_bc[:], recip[:])
                    nc.vector.tensor_mul(out=yT[h][:], in0=yps[i][0:64, :],
                                         in1=recip_bc[:])
            es_b.close()

            # ---- out-proj (token-local) + residual ----
            es_c = ExitStack()
            psO = es_c.enter_context(tc.tile_pool(name="psO", bufs=2, space="PSUM"))
            watp_sb = [watpp.tile([64, E], BF16, tag=f"watp{h}", name=f"watp{h}")
                       for h in range(H)]
            for h in range(H):
                nc.sync.dma_start(out=watp_sb[h][:], in_=watp[layer, h])
            atpb_bc = bcast_row(lnrow, atpb[layer], 128, BF16, E, "atpb")
            for t in range(2):
                ps = psO.tile([128, E], F32, tag="o", name="o_ps")
                for n0, n1 in ((0, 512), (512, 768)):
                    for h in range(H):
                        nc.tensor.matmul(ps[:, n0:n1],
                                         yT[h][:, t * 128:(t + 1) * 128],
                                         watp_sb[h][:, n0:n1],
                                         start=(h == 0), stop=(h == H - 1))
                nc.vector.tensor_add(out=x_sb[t][:], in0=x_sb[t][:], in1=ps[:])
                nc.vector.tensor_add(out=x_sb[t][:], in0=x_sb[t][:], in1=atpb_bc[:])
            es_c.close()

            unscope()
            scope(f"L{layer}_mlp")
            # ---- LN2 + transpose ----
            ln2g_bc = bcast_row(lnrow, ln2g[layer], 128, BF16, E, "ln_g")
            ln2b_bc = bcast_row(lnrow, ln2b[layer], 128, BF16, E, "ln_b")
            h2T = [hTp.tile([128, TS], BF16, tag=f"hT{k}", name=f"hT{k}") for k in range(6)]
            for t in range(2):
                h_t = hpool.tile([128, E], F32R, tag="h", name="h")
                layernorm_t(x_sb[t][:], ln2g_bc, ln2b_bc, h_t)
                transpose_to(h_t, h2T, t * 128)

            # ---- MLP fc: mT[m] = gelu(fc_w.T @ h2T + fc_b) ----
            es_d = ExitStack()
            psM = es_d.enter_context(tc.tile_pool(name="psM", bufs=2, space="PSUM"))
            psP = es_d.enter_context(tc.tile_pool(name="psP", bufs=1, space="PSUM"))
            fcb_sb = bias_p.tile([128, 24], F32, tag="fcb", name="fcb")
            nc.sync.dma_start(out=fcb_sb[:], in_=fcb[layer].transpose([1, 0]))
            fck = [fcwp.tile([128, 4 * E], BF16, tag=f"fck{k}", name=f"fck{k}")
                   for k in range(6)]
            for k in range(6):
                nc.sync.dma_start(out=fck[k][:], in_=fcw[layer, k])
            mT = [mTp.tile([128, TS], BF16, tag=f"mT{m}", name=f"mT{m}") for m in range(24)]
            for m in range(24):
                ps = psM.tile([128, TS], F32, tag="m", name="m_ps")
                for k in range(6):
                    nc.tensor.matmul(ps[:], fck[k][:, m * 128:(m + 1) * 128],
                                     h2T[k][:], start=(k == 0), stop=(k == 5))
                nc.scalar.activation(out=mT[m][:], in_=ps[:],
                                     func=mybir.ActivationFunctionType.Gelu_apprx_tanh,
                                     bias=fcb_sb[:, m:m + 1])

            # ---- MLP pr + residual ----
            prb_bc = bcast_row(lnrow, prb[layer], 128, BF16, E, "prb")
            ps2 = [psP.tile([128, E], F32, tag=f"p{t}", name=f"p{t}") for t in range(2)]
            for kk in range(24):
                prw_sb = prwp.tile([128, E], BF16, tag="prw", name="prw")
                nc.sync.dma_start(out=prw_sb[:],
                                  in_=prw[layer, kk * 128:(kk + 1) * 128, :])
                for t in range(2):
                    for n0, n1 in ((0, 512), (512, 768)):
                        nc.tensor.matmul(ps2[t][:, n0:n1],
                                         mT[kk][:, t * 128:(t + 1) * 128],
                                         prw_sb[:, n0:n1],
                                         start=(kk == 0), stop=(kk == 23))
            for t in range(2):
                nc.vector.tensor_add(out=x_sb[t][:], in0=x_sb[t][:], in1=ps2[t][:])
                nc.vector.tensor_add(out=x_sb[t][:], in0=x_sb[t][:], in1=prb_bc[:])
            es_d.close()
            unscope()

        # ---- final LN + AllGather(all 8) + lm_head ----
        scope("lnf_ag")
        lnfg_bc = bcast_row(lnrow, lnfg[0], 128, BF16, E, "ln_g")
        lnfb_bc = bcast_row(lnrow, lnfb[0], 128, BF16, E, "ln_b")
        xfT = [hTp.tile([128, TS], BF16, tag=f"hT{k}", name=f"hT{k}") for k in range(6)]
        for t in range(2):
            h_t = hpool.tile([128, E], F32R, tag="h", name="h")
            layernorm_t(x_sb[t][:], lnfg_bc, lnfb_bc, h_t)
            transpose_to(h_t, xfT, t * 128)
        for k in range(6):
            nc.sync.dma_start(out=xf_in[k * 128:(k + 1) * 128, :], in_=xfT[k][:])
        nc.gpsimd.collective_compute(
            "AllGather", mybir.AluOpType.bypass,
            replica_groups=g_all,
            ins=[xf_in.opt()],
            outs=[xf_ag.opt()],
        )
        es_l.close()
        es_h = es.enter_context(ExitStack())
        xfp = es_h.enter_context(tc.tile_pool(name="xfp", bufs=1))
        wtep = es_h.enter_context(tc.tile_pool(name="wtep", bufs=3))
        lop = es_h.enter_context(tc.tile_pool(name="lop", bufs=4))
        psL = es_h.enter_context(tc.tile_pool(name="psL", bufs=4, space="PSUM"))

        xf_sb = [xfp.tile([128, TS], BF16, tag=f"xf{i}", name=f"xf{i}") for i in range(48)]
        for i in range(48):
            nc.sync.dma_start(out=xf_sb[i][:], in_=xf_ag[i * 128:(i + 1) * 128, :])
        unscope()

        scope("lmhead")
        nch = (VS + 511) // 512
        for n in range(nch):
            n0 = n * 512
            nw = min(512, VS - n0)
            wte_sb = [wtep.tile([128, 512], BF16, tag=f"wte{k}", name=f"wte{k}")
                      for k in range(6)]
            for k in range(6):
                nc.sync.dma_start(out=wte_sb[k][:, 0:nw],
                                  in_=wteT[k * 128:(k + 1) * 128, n0:n0 + nw])
            for t in range(16):
                r, half = t // 2, t % 2
                ps = psL.tile([128, 512], F32, tag="l", name="l_ps")
                for k in range(6):
                    nc.tensor.matmul(ps[:, 0:nw],
                                     xf_sb[r * 6 + k][:, half * 128:(half + 1) * 128],
                                     wte_sb[k][:, 0:nw],
                                     start=(k == 0), stop=(k == 5))
                lo = lop.tile([128, 512], F32, tag="lo", name="lo")
                if t % 2 == 0:
                    nc.vector.tensor_copy(lo[:, 0:nw], ps[:, 0:nw])
                else:
                    nc.scalar.activation(out=lo[:, 0:nw], in_=ps[:, 0:nw],
                                         func=mybir.ActivationFunctionType.Copy)
                nc.sync.dma_start(out=logits[t * 128:(t + 1) * 128, n0:n0 + nw],
                                  in_=lo[:, 0:nw])
        unscope()

    nc.compile()
    return nc


def _prep_inputs(idx, wte, wpe, ln1_w, ln1_b, attn_w, attn_b, atp_w, atp_b,
                 ln2_w, ln2_b, fc_w, fc_b, pr_w, pr_b, lnf_w, lnf_b):
    idx = np.asarray(idx)
    f = lambda a: np.ascontiguousarray(np.asarray(a), dtype=np.float32)
    bf = lambda a: np.ascontiguousarray(np.asarray(a, dtype=np.float32).astype(BF))
    wte32, wpe32 = f(wte), f(wpe)
    x0 = wte32[idx.reshape(-1)] + np.tile(wpe32[:T], (B, 1))  # [2048, 768]
    wte_pad = np.zeros((VPAD, E), np.float32)
    wte_pad[:V] = wte32
    wteT_full = np.ascontiguousarray(wte_pad.T).astype(BF)  # [768, VPAD]

    attn_w, attn_b = f(attn_w), f(attn_b)
    common = {
        "wq": bf(attn_w[:, :, 0:E]),
        "bq": np.ascontiguousarray(attn_b[:, 0:E].reshape(L, H, HD)),
        "wk": bf(attn_w[:, :, E:2 * E]),
        "bk": np.ascontiguousarray(attn_b[:, E:2 * E].reshape(L, 6, 128)),
        "wv": bf(attn_w[:, :, 2 * E:3 * E]),
        "bv": bf(attn_b[:, 2 * E:3 * E]),
        "watp": bf(np.asarray(atp_w).reshape(L, H, HD, E)),
        "atpb": bf(atp_b),
        "fcw": bf(np.asarray(fc_w).reshape(L, 6, 128, 4 * E)),
        "fcb": np.ascontiguousarray(f(fc_b).reshape(L, 24, 128)),
        "prw": bf(pr_w), "prb": bf(pr_b),
        "ln1g": bf(ln1_w), "ln1b": bf(ln1_b),
        "ln2g": bf(ln2_w), "ln2b": bf(ln2_b),
        "lnfg": bf(lnf_w).reshape(1, E), "lnfb": bf(lnf_b).reshape(1, E),
    }
    in_maps = []
    kidx = np.arange(128)
    qidx = np.arange(TS)
    for c in range(NC):
        r = c % 4
        m = np.zeros((8, 128, TS), np.float32)
        for j in range(8):
            m[j] = ((128 * j + kidx)[:, None] <= (TS * r + qidx)[None, :])
        m2 = np.concatenate([m, m], axis=2)
        in_maps.append({
            **common,
            "x0s": np.ascontiguousarray(x0[c * TS:(c + 1) * TS]),
            "mask": m2.astype(BF),
            "wteT": np.ascontiguousarray(wteT_full[:, c * VS:(c + 1) * VS]),
        })
    return in_maps


def kernel(trace=False, **inputs):
    if "nc" not in _CACHE:
        _CACHE["nc"] = _build_program()
    nc = _CACHE["nc"]
    in_maps = _prep_inputs(**inputs)
    res = run_bass_kernel_spmd(nc, in_maps, core_ids=list(range(NC)), trace=trace)
    _CACHE["last_result"] = res
    logits = np.concatenate([res.results[c]["logits"] for c in range(NC)], axis=1)
    return logits[:, :V].reshape(B, T, V).astype(np.float32)


# revision 24
# speedup vs baseline: 1.4147x; 1.0082x over previous
"""GPT (4-layer, E=768, H=12, T=1024, B=2, V=50257) forward on 8 trn2 cores.

Sharding (v3):
  - Token-parallel residual: core c owns tokens [r*256,(r+1)*256) of batch
    c//4 (r = c%4).  LN / QKV / out-proj / MLP fully token-local.
  - Attention: per layer two AllGathers per batch group of 4 — K (dim-major
    [768,256] bf16) and V (token-major [256,780] bf16, softmax ones column
    pre-packed).  Each core computes all 12 heads for its own 256 queries
    against the full 1024 keys with a per-core causal mask (uniform SPMD
    program; masking by data, not control flow).  No ReduceScatter.
  - lm_head vocab-sharded: final AllGather of lnf(x) (transposed, bf16),
    each core computes a [2048, 6284] logit slice (V padded to 50272).
  - All matmul operands bf16 (fp32 PSUM accumulation); weights shipped bf16
    to halve HBM traffic and PE weight-load time.
"""

import sys
from contextlib import ExitStack
import numpy as np
import ml_dtypes

sys.path.insert(0, "/opt/trn_rl_repo")

import concourse.bass as bass
import concourse.mybir as mybir
import concourse.tile as tile
from concourse import bacc
from concourse.bass_utils import run_bass_kernel_spmd
from concourse.masks import make_identity

L, H, E, T, V = 4, 12, 768, 1024, 50257
B = 2
NC = 8
TS = (B * T) // NC          # 256 tokens per core
VS = 6284                   # vocab slice per core (padded V = 50272)
VPAD = VS * NC
HD = 64
EPS = 1e-5
SCALE = float(1.0 / np.sqrt(np.float32(E)))
F32 = mybir.dt.float32
F32R = mybir.dt.float32r
BF16 = mybir.dt.bfloat16
BF = ml_dtypes.bfloat16

_CACHE = {}


def _build_program():
    nc = bacc.Bacc("TRN2", target_bir_lowering=False, debug=False, num_devices=NC)

    # ---- I/O -------------------------------------------------------------
    x0s = nc.dram_tensor("x0s", [TS, E], F32, kind="ExternalInput")
    wq = nc.dram_tensor("wq", [L, E, E], BF16, kind="ExternalInput")
    bq = nc.dram_tensor("bq", [L, H, HD], F32, kind="ExternalInput")
    wk = nc.dram_tensor("wk", [L, E, E], BF16, kind="ExternalInput")
    bk = nc.dram_tensor("bk", [L, 6, 128], F32, kind="ExternalInput")
    wv = nc.dram_tensor("wv", [L, E, E], BF16, kind="ExternalInput")
    bv = nc.dram_tensor("bv", [L, E], BF16, kind="ExternalInput")
    watp = nc.dram_tensor("watp", [L, H, HD, E], BF16, kind="ExternalInput")
    atpb = nc.dram_tensor("atpb", [L, E], BF16, kind="ExternalInput")
    fcw = nc.dram_tensor("fcw", [L, 6, 128, 4 * E], BF16, kind="ExternalInput")
    fcb = nc.dram_tensor("fcb", [L, 24, 128], F32, kind="ExternalInput")
    prw = nc.dram_tensor("prw", [L, 4 * E, E], BF16, kind="ExternalInput")
    prb = nc.dram_tensor("prb", [L, E], BF16, kind="ExternalInput")
    ln1g = nc.dram_tensor("ln1g", [L, E], BF16, kind="ExternalInput")
    ln1b = nc.dram_tensor("ln1b", [L, E], BF16, kind="ExternalInput")
    ln2g = nc.dram_tensor("ln2g", [L, E], BF16, kind="ExternalInput")
    ln2b = nc.dram_tensor("ln2b", [L, E], BF16, kind="ExternalInput")
    lnfg = nc.dram_tensor("lnfg", [1, E], BF16, kind="ExternalInput")
    lnfb = nc.dram_tensor("lnfb", [1, E], BF16, kind="ExternalInput")
    mask = nc.dram_tensor("mask", [8, 128, 2 * TS], BF16, kind="ExternalInput")
    wteT = nc.dram_tensor("wteT", [E, VS], BF16, kind="ExternalInput")
    logits = nc.dram_tensor("logits", [B * T, VS], F32, kind="ExternalOutput")

    g_all = [list(range(NC))]
    g_batch = [[0, 1, 2, 3], [4, 5, 6, 7]]

    def bcast_row(pool, src_ap, n, dtype, w, name="bc"):
        """Replicate a [w] DRAM row across n partitions via broadcast DMA."""
        t = pool.tile([n, w], dtype, tag=name, name=name)
        in_ap = bass.AP(
            tensor=src_ap.tensor,
            offset=src_ap.offset,
            ap=[[0, n]] + [list(p) for p in src_ap.ap],
        )
        nc.sync.dma_start(out=t[:], in_=in_ap)
        return t

    with tile.TileContext(nc) as tc, ExitStack() as es:
        const = es.enter_context(tc.tile_pool(name="const", bufs=1))
        xp = es.enter_context(tc.tile_pool(name="xp", bufs=1))
        lnrow = es.enter_context(tc.tile_pool(name="lnrow", bufs=2))
        stat = es.enter_context(tc.tile_pool(name="stat", bufs=4))
        hpool = es.enter_context(tc.tile_pool(name="hpool", bufs=2))
        hTp = es.enter_context(tc.tile_pool(name="hTp", bufs=1))
        dram = es.enter_context(tc.tile_pool(name="dram", bufs=1, space="DRAM"))
        psT = es.enter_context(tc.tile_pool(name="psT", bufs=2, space="PSUM"))

        ident_f = const.tile([128, 128], F32, name="ident_f")
        make_identity(nc, ident_f)
        ident = const.tile([128, 128], F32R, name="ident")
        nc.vector.tensor_copy(ident[:], ident_f[:])
        eps_sb = const.tile([128, 1], F32)
        nc.vector.memset(eps_sb, EPS)
        ones_f = const.tile([128, H * 65], F32, name="ones_f")
        nc.vector.memset(ones_f, 1.0)
        ones780 = const.tile([128, H * 65], BF16, name="ones780")
        nc.vector.tensor_copy(ones780[:], ones_f[:])
        mask_sb = [const.tile([128, 2 * TS], BF16, name=f"mask{j}") for j in range(8)]
        for j in range(8):
            nc.sync.dma_start(out=mask_sb[j][:], in_=mask[j])

        # persistent residual stream [256, 768] as two [128, 768] f32 tiles
        x_sb = [xp.tile([128, E], F32, tag=f"x{t}", name=f"x{t}") for t in range(2)]
        for t in range(2):
            nc.sync.dma_start(out=x_sb[t][:], in_=x0s[t * 128:(t + 1) * 128, :])

        # DRAM bounce buffers for collectives
        kT_in = dram.tile([E, TS], BF16)
        kT_ag = dram.tile([4 * E, TS], BF16)
        v_in = dram.tile([TS, H * 65], BF16)
        v_ag = dram.tile([4 * TS, H * 65], BF16)
        xf_in = dram.tile([E, TS], BF16)
        xf_ag = dram.tile([NC * E, TS], BF16, addr_space="Shared")

        def layernorm_t(x_ap, g_bc, b_bc, out_tile):
            """LN over free dim (768) of [128, 768] f32 tile."""
            stats = stat.tile([128, 3, 6], F32, tag="bn_stats", name="bn_stats_t")
            xr = x_ap.rearrange("p (s d) -> p s d", s=3)
            for s in range(3):
                nc.vector.bn_stats(out=stats[:, s, :], in_=xr[:, s, :])
            mv = stat.tile([128, 2], F32, tag="bn_aggr", name="bn_aggr_t")
            nc.vector.bn_aggr(out=mv[:], in_=stats[:])
            rstd = stat.tile([128, 1], F32, tag="rstd", name="rstd_t")
            nc.scalar.activation(out=rstd[:], in_=mv[:, 1:2],
                                 func=mybir.ActivationFunctionType.Sqrt,
                                 bias=eps_sb[:], scale=1.0)
            nc.vector.reciprocal(out=rstd[:], in_=rstd[:])
            nc.vector.tensor_scalar(out=out_tile[:], in0=x_ap,
                                    scalar1=mv[:, 0:1], scalar2=rstd[:],
                                    op0=mybir.AluOpType.subtract,
                                    op1=mybir.AluOpType.mult)
            nc.vector.tensor_mul(out=out_tile[:], in0=out_tile[:], in1=g_bc[:])
            nc.vector.tensor_add(out=out_tile[:], in0=out_tile[:], in1=b_bc[:])

        def transpose_to(hsrc, dst_tiles, dst_col):
            """hsrc [128,768] f32r -> bf16 dst_tiles[k][:, dst_col:dst_col+128]."""
            for k in range(6):
                pt = psT.tile([128, 128], F32R, tag="tr", name="tr")
                nc.tensor.transpose(pt[:], hsrc[:, k * 128:(k + 1) * 128], ident[:])
                dst = dst_tiles[k][:, dst_col:dst_col + 128]
                if k % 2 == 0:
                    nc.vector.tensor_copy(dst, pt[:].bitcast(F32))
                else:
                    nc.scalar.activation(out=dst, in_=pt[:].bitcast(F32),
                                         func=mybir.ActivationFunctionType.Copy)

        def scope(name):
            sid, _ = nc.enter_named_scope(name, False)
            _SCOPES.append((name, sid))

        def unscope():
            name, sid = _SCOPES.pop()
            nc.leave_named_scope(name, sid, False)

        _SCOPES = []

        es_l = es.enter_context(ExitStack())
        wqp = es_l.enter_context(tc.tile_pool(name="wqp", bufs=1))
        wkp = es_l.enter_context(tc.tile_pool(name="wkp", bufs=1))
        wvp = es_l.enter_context(tc.tile_pool(name="wvp", bufs=1))
        watpp = es_l.enter_context(tc.tile_pool(name="watpp", bufs=1))
        bias_p = es_l.enter_context(tc.tile_pool(name="bias_p", bufs=2))
        qTp = es_l.enter_context(tc.tile_pool(name="qTp", bufs=1))
        kvsp = es_l.enter_context(tc.tile_pool(name="kvsp", bufs=2))
        kTp = es_l.enter_context(tc.tile_pool(name="kTp", bufs=1))
        vp = es_l.enter_context(tc.tile_pool(name="vp", bufs=1))
        ep = es_l.enter_context(tc.tile_pool(name="ep", bufs=3))
        yp = es_l.enter_context(tc.tile_pool(name="yp", bufs=1))
        sm = es_l.enter_context(tc.tile_pool(name="sm", bufs=4))
        fcwp = es_l.enter_context(tc.tile_pool(name="fcwp", bufs=1))
        mTp = es_l.enter_context(tc.tile_pool(name="mTp", bufs=1))
        prwp = es_l.enter_context(tc.tile_pool(name="prwp", bufs=3))

        for layer in range(L):
            scope(f"L{layer}_qkv")
            ln1g_bc = bcast_row(lnrow, ln1g[layer], 128, BF16, E, "ln_g")
            ln1b_bc = bcast_row(lnrow, ln1b[layer], 128, BF16, E, "ln_b")

            # ---- LN1 + transpose -> hT [768, 256] bf16 ----
            hT = [hTp.tile([128, TS], BF16, tag=f"hT{k}", name=f"hT{k}") for k in range(6)]
            for t in range(2):
                h_t = hpool.tile([128, E], F32R, tag="h", name="h")
                layernorm_t(x_sb[t][:], ln1g_bc, ln1b_bc, h_t)
                transpose_to(h_t, hT, t * 128)

            # ---- K (dim-major, per-partition bias) ----
            wk_sb = [wkp.tile([128, E], BF16, tag=f"wk{k}", name=f"wk{k}")
                     for k in range(6)]
            for k in range(6):
                nc.sync.dma_start(out=wk_sb[k][:],
                                  in_=wk[layer, k * 128:(k + 1) * 128, :])
            bk_sb = bias_p.tile([128, 6], F32, tag="bk", name="bk")
            nc.sync.dma_start(out=bk_sb[:], in_=bk[layer].transpose([1, 0]))
            es_a = ExitStack()
            psK = es_a.enter_context(tc.tile_pool(name="psK", bufs=2, space="PSUM"))
            psV = es_a.enter_context(tc.tile_pool(name="psV", bufs=1, space="PSUM"))
            for m in range(6):
                ps = psK.tile([128, TS], F32, tag="kps", name="k_ps")
                for k in range(6):
                    nc.tensor.matmul(ps[:], wk_sb[k][:, m * 128:(m + 1) * 128],
                                     hT[k][:], start=(k == 0), stop=(k == 5))
                kt = kvsp.tile([128, TS], BF16, tag="kloc", name="kloc")
                nc.vector.tensor_scalar_add(out=kt[:], in0=ps[:],
                                            scalar1=bk_sb[:, m:m + 1])
                nc.sync.dma_start(out=kT_in[m * 128:(m + 1) * 128, :], in_=kt[:])
            nc.gpsimd.collective_compute(
                "AllGather", mybir.AluOpType.bypass,
                replica_groups=g_batch,
                ins=[kT_in.opt()],
                outs=[kT_ag.opt()],
            )

            # ---- V (token-major, ones column pre-packed) ----
            wv_sb = [wvp.tile([128, E], BF16, tag=f"wv{k}", name=f"wv{k}")
                     for k in range(6)]
            for k in range(6):
                nc.sync.dma_start(out=wv_sb[k][:],
                                  in_=wv[layer, k * 128:(k + 1) * 128, :])
            bv_bc = bcast_row(lnrow, bv[layer], 128, BF16, E, "bv")
            for t in range(2):
                ps = psV.tile([128, E], F32, tag="vps", name="v_ps")
                for n0, n1 in ((0, 512), (512, 768)):
                    for k in range(6):
                        nc.tensor.matmul(ps[:, n0:n1],
                                         hT[k][:, t * 128:(t + 1) * 128],
                                         wv_sb[k][:, n0:n1],
                                         start=(k == 0), stop=(k == 5))
                vt = kvsp.tile([128, H * 65], BF16, tag="vloc", name="vloc")
                nc.vector.tensor_copy(vt[:], ones780[:])
                for h in range(H):
                    nc.vector.tensor_add(out=vt[:, 65 * h:65 * h + 64],
                                         in0=ps[:, 64 * h:64 * h + 64],
                                         in1=bv_bc[:, 64 * h:64 * h + 64])
                nc.sync.dma_start(out=v_in[t * 128:(t + 1) * 128, :], in_=vt[:])
            nc.gpsimd.collective_compute(
                "AllGather", mybir.AluOpType.bypass,
                replica_groups=g_batch,
                ins=[v_in.opt()],
                outs=[v_ag.opt()],
            )

            # ---- prefetch proj/MLP weights into the AllGather window ----
            watp_sb = [watpp.tile([64, E], BF16, tag=f"watp{h}", name=f"watp{h}")
                       for h in range(H)]
            for h in range(H):
                nc.sync.dma_start(out=watp_sb[h][:], in_=watp[layer, h])
            fck = [fcwp.tile([128, 4 * E], BF16, tag=f"fck{k}", name=f"fck{k}")
                   for k in range(6)]
            for k in range(6):
                nc.sync.dma_start(out=fck[k][:], in_=fcw[layer, k])
            fcb_sb = bias_p.tile([128, 24], F32, tag="fcb", name="fcb")
            nc.sync.dma_start(out=fcb_sb[:], in_=fcb[layer].transpose([1, 0]))

            # ---- Q per head (overlaps the AllGathers) ----
            wq_sb = [wqp.tile([128, E], BF16, tag=f"wq{k}", name=f"wq{k}")
                     for k in range(6)]
            for k in range(6):
                nc.sync.dma_start(out=wq_sb[k][:],
                                  in_=wq[layer, k * 128:(k + 1) * 128, :])
            bq_sb = bias_p.tile([64, H], F32, tag="bq", name="bq")
            nc.sync.dma_start(out=bq_sb[:], in_=bq[layer].transpose([1, 0]))
            psQ = es_a.enter_context(tc.tile_pool(name="psQ", bufs=2, space="PSUM"))
            qT = [qTp.tile([64, TS], BF16, tag=f"qT{h}", name=f"qT{h}") for h in range(H)]
            for h in range(H):
                ps = psQ.tile([64, TS], F32, tag="q", name="q_ps")
                for k in range(6):
                    nc.tensor.matmul(ps[:], wq_sb[k][:, h * 64:(h + 1) * 64],
                                     hT[k][:], start=(k == 0), stop=(k == 5))
                nc.vector.tensor_scalar_add(out=qT[h][:], in0=ps[:],
                                            scalar1=bq_sb[:, h:h + 1])
            es_a.close()

            unscope()
            scope(f"L{layer}_attn")
            # ---- load gathered K (per-head, base-0) and V (plain) ----
            kTh = [kTp.tile([64, T], BF16, tag=f"kTh{h}", name=f"kTh{h}")
                   for h in range(H)]
            kT_view = kT_ag[:].rearrange("(r m q d) c -> r m q d c", r=4, m=6, q=2)
            for h in range(H):
                nc.sync.dma_start(
                    out=kTh[h][:].rearrange("p (r c) -> p r c", r=4),
                    in_=kT_view[:, h // 2, h % 2].transpose([1, 0, 2]))
            v_sb = [vp.tile([128, H * 65], BF16, tag=f"v{j}", name=f"v{j}")
                    for j in range(8)]
            for j in range(8):
                nc.sync.dma_start(out=v_sb[j][:],
                                  in_=v_ag[j * 128:(j + 1) * 128, :])

            # ---- attention: 12 heads, 256 queries vs 1024 masked keys ----
            es_b = ExitStack()
            psS = es_b.enter_context(tc.tile_pool(name="psS", bufs=2, space="PSUM"))
            psY = es_b.enter_context(tc.tile_pool(name="psY", bufs=2, space="PSUM"))
            yT = [yp.tile([64, TS], BF16, tag=f"yT{h}", name=f"yT{h}") for h in range(H)]
            for hp in range(6):
                yps = [psY.tile([65, TS], F32, tag=f"y{i}", name=f"y{i}")
                       for i in range(2)]
                for j in range(8):
                    pss = psS.tile([128, 2 * TS], F32, tag="s", name="s_ps")
                    e_sb = ep.tile([128, 2 * TS], BF16, tag="e", name="e")
                    for i in range(2):
                        nc.tensor.matmul(pss[:, i * TS:(i + 1) * TS],
                                         kTh[2 * hp + i][:, j * 128:(j + 1) * 128],
                                         qT[2 * hp + i][:], start=True, stop=True)
                    nc.scalar.activation(out=e_sb[:], in_=pss[:],
                                         func=mybir.ActivationFunctionType.Exp,
                                         scale=SCALE)
                    nc.vector.tensor_mul(out=e_sb[:], in0=e_sb[:], in1=mask_sb[j][:])
                    for i in range(2):
                        h = 2 * hp + i
                        nc.tensor.matmul(yps[i][:], v_sb[j][:, 65 * h:65 * h + 65],
                                         e_sb[:, i * TS:(i + 1) * TS],
                                         start=(j == 0), stop=(j == 7))
                for i in range(2):
                    h = 2 * hp + i
                    recip = sm.tile([1, TS], F32, tag="recip", name="recip")
                    nc.vector.reciprocal(out=recip[:], in_=yps[i][64:65, :])
                    recip_bc = sm.tile([64, TS], F32, tag="recip_bc", name="recip_bc")
                    nc.gpsimd.partition_broadcast(recip_bc[:], recip[:])
                    nc.vector.tensor_mul(out=yT[h][:], in0=yps[i][0:64, :],
                                         in1=recip_bc[:])
            es_b.close()

            # ---- out-proj (token-local) + residual ----
            es_c = ExitStack()
            psO = es_c.enter_context(tc.tile_pool(name="psO", bufs=2, space="PSUM"))
            atpb_bc = bcast_row(lnrow, atpb[layer], 128, BF16, E, "atpb")
            for t in range(2):
                ps = psO.tile([128, E], F32, tag="o", name="o_ps")
                for n0, n1 in ((0, 512), (512, 768)):
                    for h in range(H):
                        nc.tensor.matmul(ps[:, n0:n1],
                                         yT[h][:, t * 128:(t + 1) * 128],
                                         watp_sb[h][:, n0:n1],
                                         start=(h == 0), stop=(h == H - 1))
                nc.vector.tensor_add(out=x_sb[t][:], in0=x_sb[t][:], in1=ps[:])
                nc.vector.tensor_add(out=x_sb[t][:], in0=x_sb[t][:], in1=atpb_bc[:])
            es_c.close()

            unscope()
            scope(f"L{layer}_mlp")
            # ---- LN2 + transpose ----
            ln2g_bc = bcast_row(lnrow, ln2g[layer], 128, BF16, E, "ln_g")
            ln2b_bc = bcast_row(lnrow, ln2b[layer], 128, BF16, E, "ln_b")
            h2T = [hTp.tile([128, TS], BF16, tag=f"hT{k}", name=f"hT{k}") for k in range(6)]
            for t in range(2):
                h_t = hpool.tile([128, E], F32R, tag="h", name="h")
                layernorm_t(x_sb[t][:], ln2g_bc, ln2b_bc, h_t)
                transpose_to(h_t, h2T, t * 128)

            # ---- MLP fc: mT[m] = gelu(fc_w.T @ h2T + fc_b) ----
            es_d = ExitStack()
            psM = es_d.enter_context(tc.tile_pool(name="psM", bufs=2, space="PSUM"))
            psP = es_d.enter_context(tc.tile_pool(name="psP", bufs=1, space="PSUM"))
            mT = [mTp.tile([128, TS], BF16, tag=f"mT{m}", name=f"mT{m}") for m in range(24)]
            for m in range(24):
                ps = psM.tile([128, TS], F32, tag="m", name="m_ps")
                for k in range(6):
                    nc.tensor.matmul(ps[:], fck[k][:, m * 128:(m + 1) * 128],
                                     h2T[k][:], start=(k == 0), stop=(k == 5))
                nc.scalar.activation(out=mT[m][:], in_=ps[:],
                                     func=mybir.ActivationFunctionType.Gelu_apprx_tanh,
                                     bias=fcb_sb[:, m:m + 1])

            # ---- MLP pr + residual ----
            prb_bc = bcast_row(lnrow, prb[layer], 128, BF16, E, "prb")
            ps2 = [psP.tile([128, E], F32, tag=f"p{t}", name=f"p{t}") for t in range(2)]
            for kk in range(24):
                prw_sb = prwp.tile([128, E], BF16, tag="prw", name="prw")
                nc.sync.dma_start(out=prw_sb[:],
                                  in_=prw[layer, kk * 128:(kk + 1) * 128, :])
                for t in range(2):
                    for n0, n1 in ((0, 512), (512, 768)):
                        nc.tensor.matmul(ps2[t][:, n0:n1],
                                         mT[kk][:, t * 128:(t + 1) * 128],
                                         prw_sb[:, n0:n1],
                                         start=(kk == 0), stop=(kk == 23))
            for t in range(2):
                nc.vector.tensor_add(out=x_sb[t][:], in0=x_sb[t][:], in1=ps2[t][:])
                nc.vector.tensor_add(out=x_sb[t][:], in0=x_sb[t][:], in1=prb_bc[:])
            es_d.close()
            unscope()

        # ---- final LN + AllGather(all 8) + lm_head ----
        scope("lnf_ag")
        lnfg_bc = bcast_row(lnrow, lnfg[0], 128, BF16, E, "ln_g")
        lnfb_bc = bcast_row(lnrow, lnfb[0], 128, BF16, E, "ln_b")
        xfT = [hTp.tile([128, TS], BF16, tag=f"hT{k}", name=f"hT{k}") for k in range(6)]
        for t in range(2):
            h_t = hpool.tile([128, E], F32R, tag="h", name="h")
            layernorm_t(x_sb[t][:], lnfg_bc, lnfb_bc, h_t)
            transpose_to(h_t, xfT, t * 128)
        for k in range(6):
            nc.sync.dma_start(out=xf_in[k * 128:(k + 1) * 128, :], in_=xfT[k][:])
        nc.gpsimd.collective_compute(
            "AllGather", mybir.AluOpType.bypass,
            replica_groups=g_all,
            ins=[xf_in.opt()],
            outs=[xf_ag.opt()],
        )
        es_l.close()
        es_h = es.enter_context(ExitStack())
        xfp = es_h.enter_context(tc.tile_pool(name="xfp", bufs=1))
        wtep = es_h.enter_context(tc.tile_pool(name="wtep", bufs=3))
        lop = es_h.enter_context(tc.tile_pool(name="lop", bufs=4))
        psL = es_h.enter_context(tc.tile_pool(name="psL", bufs=4, space="PSUM"))

        xf_sb = [xfp.tile([128, TS], BF16, tag=f"xf{i}", name=f"xf{i}") for i in range(48)]
        for i in range(48):
            nc.sync.dma_start(out=xf_sb[i][:], in_=xf_ag[i * 128:(i + 1) * 128, :])
        unscope()

        scope("lmhead")
        nch = (VS + 511) // 512
        for n in range(nch):
            n0 = n * 512
            nw = min(512, VS - n0)
            wte_sb = [wtep.tile([128, 512], BF16, tag=f"wte{k}", name=f"wte{k}")
                      for k in range(6)]
            for k in range(6):
                nc.sync.dma_start(out=wte_sb[k][:, 0:nw],
                                  in_=wteT[k * 128:(k + 1) * 128, n0:n0 + nw])
            for t in range(16):
                r, half = t // 2, t % 2
                ps = psL.tile([128, 512], F32, tag="l", name="l_ps")
                for k in range(6):
                    nc.tensor.matmul(ps[:, 0:nw],
                                     xf_sb[r * 6 + k][:, half * 128:(half + 1) * 128],
                                     wte_sb[k][:, 0:nw],
                                     start=(k == 0), stop=(k == 5))
                lo = lop.tile([128, 512], F32, tag="lo", name="lo")
                if t % 2 == 0:
                    nc.vector.tensor_copy(lo[:, 0:nw], ps[:, 0:nw])
                else:
                    nc.scalar.activation(out=lo[:, 0:nw], in_=ps[:, 0:nw],
                                         func=mybir.ActivationFunctionType.Copy)
                nc.sync.dma_start(out=logits[t * 128:(t + 1) * 128, n0:n0 + nw],
                                  in_=lo[:, 0:nw])
        unscope()

    nc.compile()
    return nc


def _prep_inputs(idx, wte, wpe, ln1_w, ln1_b, attn_w, attn_b, atp_w, atp_b,
                 ln2_w, ln2_b, fc_w, fc_b, pr_w, pr_b, lnf_w, lnf_b):
    idx = np.asarray(idx)
    f = lambda a: np.ascontiguousarray(np.asarray(a), dtype=np.float32)
    bf = lambda a: np.ascontiguousarray(np.asarray(a, dtype=np.float32).astype(BF))
    wte32, wpe32 = f(wte), f(wpe)
    x0 = wte32[idx.reshape(-1)] + np.tile(wpe32[:T], (B, 1))  # [2048, 768]
    wte_pad = np.zeros((VPAD, E), np.float32)
    wte_pad[:V] = wte32
    wteT_full = np.ascontiguousarray(wte_pad.T).astype(BF)  # [768, VPAD]

    attn_w, attn_b = f(attn_w), f(attn_b)
    common = {
        "wq": bf(attn_w[:, :, 0:E]),
        "bq": np.ascontiguousarray(attn_b[:, 0:E].reshape(L, H, HD)),
        "wk": bf(attn_w[:, :, E:2 * E]),
        "bk": np.ascontiguousarray(attn_b[:, E:2 * E].reshape(L, 6, 128)),
        "wv": bf(attn_w[:, :, 2 * E:3 * E]),
        "bv": bf(attn_b[:, 2 * E:3 * E]),
        "watp": bf(np.asarray(atp_w).reshape(L, H, HD, E)),
        "atpb": bf(atp_b),
        "fcw": bf(np.asarray(fc_w).reshape(L, 6, 128, 4 * E)),
        "fcb": np.ascontiguousarray(f(fc_b).reshape(L, 24, 128)),
        "prw": bf(pr_w), "prb": bf(pr_b),
        "ln1g": bf(ln1_w), "ln1b": bf(ln1_b),
        "ln2g": bf(ln2_w), "ln2b": bf(ln2_b),
        "lnfg": bf(lnf_w).reshape(1, E), "lnfb": bf(lnf_b).reshape(1, E),
    }
    in_maps = []
    kidx = np.arange(128)
    qidx = np.arange(TS)
    for c in range(NC):
        r = c % 4
        m = np.zeros((8, 128, TS), np.float32)
        for j in range(8):
            m[j] = ((128 * j + kidx)[:, None] <= (TS * r + qidx)[None, :])
        m2 = np.concatenate([m, m], axis=2)
        in_maps.append({
            **common,
            "x0s": np.ascontiguousarray(x0[c * TS:(c + 1) * TS]),
            "mask": m2.astype(BF),
            "wteT": np.ascontiguousarray(wteT_full[:, c * VS:(c + 1) * VS]),
        })
    return in_maps


def kernel(trace=False, **inputs):
    if "nc" not in _CACHE:
        _CACHE["nc"] = _build_program()
    nc = _CACHE["nc"]
    in_maps = _prep_inputs(**inputs)
    res = run_bass_kernel_spmd(nc, in_maps, core_ids=list(range(NC)), trace=trace)
    _CACHE["last_result"] = res
    logits = np.concatenate([res.results[c]["logits"] for c in range(NC)], axis=1)
    return logits[:, :V].reshape(B, T, V).astype(np.float32)
